# revision 1
# baseline (speedup 1.0000x reference)
"""Trainium2 Bass kernel for nn_BaseLayerGate (MoE balanced routing).

8 NeuronCores, data-parallel over tokens:
  - Each core owns a 2048-token shard. Affinity matmul aff^T = centT.T @ featsT
    on the tensor engine (fp32): col-major aff^T [128 (2 slots x 64 experts), 2048].
  - Sinkhorn (10 iters) in reciprocal-potential form:
      R_sum[n]  = sum_se E0[n,se] * V[se]      (PE matvec, V = slot-masked 1/C_sum)
      C_sum[se] = sum_n  E0[n,se] * W[n]       (PE matvec accum, W = 1/R_sum)
    The token-direction sum is global: per-expert partials are exchanged with an
    AllGather of a [1,128] row (PE-transposed so both exchange DMAs are
    contiguous), summed on-chip. 10 R-steps, 9 C-steps/exchanges (the 10th
    C-step is a uniform per-column shift and cannot change top-k ordering).
  - Z^T = aff^T - ln(R_sum) broadcast (ACT Ln + one Newton step for the LUT),
    per-column ordering of Z equals the reference's final ordering.
"""

import numpy as np

import concourse.bass as bass
from concourse import mybir
from concourse.bass_utils import run_bass_kernel_spmd

N_CORES = 8
N = 16384
D = 1024
KSLOT = 2
E = 64
SE = KSLOT * E
CAP = N // E
TOK = N // N_CORES
ITERS = 10

F32 = mybir.dt.float32

# ---- semaphore schedule ----------------------------------------------------
# in_sem: centT(8) + v0 + ident + ones = 11 transfers; fsem[k]: featsT chunk k
# out_sem: afft + zt outputs
# dma_sem (all x16): exchange cc_in(it) -> 2it+1, gath(it) -> 2it+2; rflat +2
D_EXCH_END = 2 * (ITERS - 1)                 # 18
D_RFLAT = D_EXCH_END + 2                     # 20
# pe_sem: 1 aff | 2 R(0) | 3..18 transposes | C(it)=19+2it, R(it>=1)=18+2it
def P_R(it):
    return 2 if it == 0 else 18 + 2 * it
def P_C(it):
    return 19 + 2 * it
P_LAST_R = P_R(ITERS - 1)                     # 36
P_ZB = P_LAST_R + 3                           # 39
# act_sem: 1 aff-copy | 2 exp | e0tm copy t -> t+3 (-> 18) | Ln | texp | rT x2
A_E0TM = 18
A_LN = A_E0TM + 1                             # 19
A_TEXP = A_LN + 1                             # 20
A_RT = A_TEXP + 2                             # 22
# dve_sem: 1 affT copy | per it<9: W=2+2it, VU=3+2it | u, rl2, sub
def V_W(it):
    return 4 * it + 1
def V_EX(it):
    return 4 * it + 2
def V_RD(it):
    return 4 * it + 3
def V_VU(it):
    return 4 * it + 4
V_U = 4 * (ITERS - 1) + 1                     # 37
V_RL2 = V_U + 1                               # 38
V_SUB = V_RL2 + 1                             # 39


def _build_nc():
    nc = bass.Bass()

    featsT_in = nc.declare_dram_parameter("featsT", [D, TOK], F32, isOutput=False)
    centT_in = nc.declare_dram_parameter("centT", [D, SE], F32, isOutput=False)
    v0_in = nc.declare_dram_parameter("v0", [SE, 2], F32, isOutput=False)
    ident_in = nc.declare_dram_parameter("ident", [128, 128], F32, isOutput=False)
    ones_in = nc.declare_dram_parameter("ones", [1, 64], F32, isOutput=False)
    onesc_in = nc.declare_dram_parameter("onesc", [128, 1], F32, isOutput=False)

    zt_out = nc.declare_dram_parameter("zt", [SE, TOK], F32, isOutput=True)
    aff_out = nc.declare_dram_parameter("afft", [SE, TOK], F32, isOutput=True)

    cc_in = nc.dram_tensor("cc_in", [SE, 1], F32)
    cc_out = nc.dram_tensor("cc_out", [N_CORES * SE, 1], F32, addr_space="Shared")

    core_ids = list(range(N_CORES))

    from contextlib import ExitStack
    es = ExitStack()
    featsT_sb = es.enter_context(nc.sbuf_tensor("featsT_sb", [128, 8, TOK], F32))
    centT_sb = es.enter_context(nc.sbuf_tensor("centT_sb", [128, 8, SE], F32))
    affT_sb = es.enter_context(nc.sbuf_tensor("affT_sb", [128, TOK], F32))
    e0t_sb = es.enter_context(nc.sbuf_tensor("e0t_sb", [128, TOK], F32))
    e0tm_sb = es.enter_context(nc.sbuf_tensor("e0tm_sb", [128, 16, 128], F32))
    ident_sb = es.enter_context(nc.sbuf_tensor("ident_sb", [128, 128], F32))
    v_sb = es.enter_context(nc.sbuf_tensor("v_sb", [128, 2], F32))
    w_sb = es.enter_context(nc.sbuf_tensor("w_sb", [128, 16, 2], F32))
    cpart_sb = es.enter_context(nc.sbuf_tensor("cpart_sb", [128, 1], F32))
    crow_sb = es.enter_context(nc.sbuf_tensor("crow_sb", [1, SE], F32))
    gath_sb = es.enter_context(nc.sbuf_tensor("gath_sb", [128, SE], F32))
    csum_sb = es.enter_context(nc.sbuf_tensor("csum_sb", [128, 1], F32))
    g8_sb = es.enter_context(nc.sbuf_tensor("g8_sb", [128, 8], F32))
    rlog_sb = es.enter_context(nc.sbuf_tensor("rlog_sb", [128, 16, 2], F32))
    texp_sb = es.enter_context(nc.sbuf_tensor("texp_sb", [128, 32], F32))
    u_sb = es.enter_context(nc.sbuf_tensor("u_sb", [128, 32], F32))
    rlog2_sb = es.enter_context(nc.sbuf_tensor("rlog2_sb", [128, 16, 2], F32))
    rt_sb = es.enter_context(nc.sbuf_tensor("rt_sb", [16, 2, 128], F32))
    rflat0_sb = es.enter_context(nc.sbuf_tensor("rflat0_sb", [1, TOK], F32))
    rflat1_sb = es.enter_context(nc.sbuf_tensor("rflat1_sb", [1, TOK], F32))
    ones_sb = es.enter_context(nc.sbuf_tensor("ones_sb", [1, 64], F32))
    onesc_sb = es.enter_context(nc.sbuf_tensor("onesc_sb", [128, 1], F32))
    zt_sb = es.enter_context(nc.sbuf_tensor("zt_sb", [128, TOK], F32))
    ps_aff = es.enter_context(nc.psum_tensor("ps_aff", [128, TOK], F32))
    ps_tp = es.enter_context(nc.psum_tensor("ps_tp", [128, 512], F32))
    ps_r = es.enter_context(nc.psum_tensor("ps_r", [128, 512], F32))
    ps_c = es.enter_context(nc.psum_tensor("ps_c", [128, 512], F32))
    ps_tp2 = es.enter_context(nc.psum_tensor("ps_tp2", [128, 512], F32))
    block = es.enter_context(nc.Block())
    dma_sem = es.enter_context(nc.semaphore("dma_sem"))
    in_sem = es.enter_context(nc.semaphore("in_sem"))
    out_sem = es.enter_context(nc.semaphore("out_sem"))
    fsems = [es.enter_context(nc.semaphore(f"fsem{k}")) for k in range(8)]
    pe_sem = es.enter_context(nc.semaphore("pe_sem"))
    act_sem = es.enter_context(nc.semaphore("act_sem"))
    dve_sem = es.enter_context(nc.semaphore("dve_sem"))
    cc_sem = es.enter_context(nc.semaphore("cc_sem"))
    with es:
        # ---------------- sync engine: all DMA ----------------
        @block.sync
        def _(eng):
            for k in range(8):
                eng.dma_start(
                    out=centT_sb[:, k, :], in_=centT_in[128 * k : 128 * (k + 1), :]
                ).then_inc(in_sem, 16)
            eng.dma_start(out=v_sb[:], in_=v0_in[:]).then_inc(in_sem, 16)
            eng.dma_start(out=ident_sb[:], in_=ident_in[:]).then_inc(in_sem, 16)
            eng.dma_start(out=ones_sb[:], in_=ones_in[:]).then_inc(in_sem, 16)
            eng.dma_start(out=onesc_sb[:], in_=onesc_in[:]).then_inc(in_sem, 16)
            for k in (0, 2, 4, 6):
                eng.dma_start(
                    out=featsT_sb[:, k, :], in_=featsT_in[128 * k : 128 * (k + 1), :]
                ).then_inc(fsems[k], 16)

            eng.wait_ge(act_sem, 1)
            eng.dma_start(out=aff_out[:], in_=affT_sb[:]).then_inc(out_sem, 16)

            for it in range(ITERS - 1):
                eng.wait_ge(dve_sem, V_EX(it))
                eng.dma_start(out=cc_in[:], in_=cpart_sb[:]).then_inc(dma_sem, 16)
                eng.wait_ge(cc_sem, it + 1)
                src_ap = cc_out.ap().rearrange("(r e) o -> e (r o)", r=N_CORES)
                with nc.allow_non_contiguous_dma(reason="8x4B strided rank gather per partition"):
                    eng.dma_start(out=g8_sb[:], in_=src_ap).then_inc(dma_sem, 16)

            eng.wait_ge(act_sem, A_RT - 1)
            dst0 = rflat0_sb.ap()[0:1].rearrange("o (t p) -> o t p", p=128)
            eng.dma_start(out=dst0, in_=rt_sb[:, 0, :]).then_inc(dma_sem, 16)

            eng.wait_ge(dve_sem, V_SUB)
            eng.dma_start(out=zt_out[:], in_=zt_sb[:]).then_inc(out_sem, 16)
            eng.wait_ge(out_sem, 32)
            eng.wait_ge(dma_sem, 16 * D_RFLAT)

        # ---------------- tensor engine ----------------
        @block.tensor
        def _(eng):
            eng.wait_ge(in_sem, 16 * 12)
            for k in range(8):
                eng.wait_ge(fsems[k], 16)
                for n in range(4):
                    mm = eng.matmul(
                        ps_aff[:, 512 * n : 512 * (n + 1)],
                        centT_sb[:, k, :],
                        featsT_sb[:, k, 512 * n : 512 * (n + 1)],
                        start=(k == 0),
                        stop=(k == 7),
                    )
            mm.then_inc(pe_sem, 1)

            # iteration-0 R-step right after exp (E0_tm not needed for it)
            eng.wait_ge(act_sem, 2)
            for t in range(16):
                mm = eng.matmul(
                    ps_r[:, 2 * t : 2 * (t + 1)],
                    e0t_sb[:, 128 * t : 128 * (t + 1)],
                    v_sb[:],
                    start=True,
                    stop=True,
                )
            mm.then_inc(pe_sem, 1)

            for t in range(16):
                if t >= 2:
                    eng.wait_ge(act_sem, t + 1)  # copy t-2 freed this buffer
                buf = ps_tp if t % 2 == 0 else ps_tp2
                eng.transpose(
                    buf[:, 0:128], e0t_sb[:, 128 * t : 128 * (t + 1)], ident_sb[:]
                ).then_inc(pe_sem, 1)

            for it in range(ITERS):
                if it > 0:
                    eng.wait_ge(dve_sem, V_VU(it - 1))
                    for t in range(16):
                        mm = eng.matmul(
                            ps_r[:, 2 * t : 2 * (t + 1)],
                            e0t_sb[:, 128 * t : 128 * (t + 1)],
                            v_sb[:],
                            start=True,
                            stop=True,
                        )
                    mm.then_inc(pe_sem, 1)

                if it < ITERS - 1:
                    if it == 0:
                        eng.wait_ge(act_sem, A_E0TM)  # all e0tm copies landed
                    eng.wait_ge(dve_sem, V_W(it))
                    for t in range(16):
                        mm = eng.matmul(
                            ps_c[:, 0:2],
                            e0tm_sb[:, t, :],
                            w_sb[:, t, :],
                            start=(t == 0),
                            stop=(t == 15),
                        )
                    mm.then_inc(pe_sem, 1)

            eng.wait_ge(dve_sem, V_RL2)
            for s in range(2):
                eng.transpose(ps_tp[0:16, 0:128], rlog2_sb[:, :, s], ident_sb[:]).then_inc(pe_sem, 1)
                eng.wait_ge(act_sem, A_TEXP + 1 + s)  # ACT copied ps_tp before reuse

            eng.wait_ge(dma_sem, 16 * D_RFLAT)
            for s in range(2):
                rsrc = rflat0_sb if s == 0 else rflat1_sb
                for n in range(4):
                    mm = eng.matmul(
                        ps_aff[64 * s : 64 * (s + 1), 512 * n : 512 * (n + 1)],
                        ones_sb[0:1, :],
                        rsrc[0:1, 512 * n : 512 * (n + 1)],
                        start=True,
                        stop=True,
                    )
            mm.then_inc(pe_sem, 1)

        # ---------------- scalar (ACT) engine ----------------
        @block.scalar
        def _(eng):
            for k in (1, 3, 5, 7):
                eng.dma_start(
                    out=featsT_sb[:, k, :], in_=featsT_in[128 * k : 128 * (k + 1), :]
                ).then_inc(fsems[k], 16)
            eng.wait_ge(pe_sem, 1)
            eng.activation(affT_sb[:], ps_aff[:, 0:TOK], mybir.ActivationFunctionType.Copy).then_inc(act_sem, 1)
            eng.wait_ge(act_sem, 1)
            eng.activation(e0t_sb[:], affT_sb[:], mybir.ActivationFunctionType.Exp).then_inc(act_sem, 1)
            for t in range(16):
                eng.wait_ge(pe_sem, 3 + t)
                buf = ps_tp if t % 2 == 0 else ps_tp2
                eng.activation(
                    e0tm_sb[:, t, :], buf[:, 0:128], mybir.ActivationFunctionType.Copy
                ).then_inc(act_sem, 1)
            eng.wait_ge(pe_sem, P_LAST_R)
            eng.activation(
                rlog_sb.ap().rearrange("p t s -> p (t s)"),
                ps_r[:, 0:32],
                mybir.ActivationFunctionType.Ln,
            ).then_inc(act_sem, 1)
            eng.wait_ge(act_sem, A_LN)
            eng.activation(
                texp_sb[:],
                rlog_sb.ap().rearrange("p t s -> p (t s)"),
                mybir.ActivationFunctionType.Exp,
                scale=-1.0,
            ).then_inc(act_sem, 1)
            for s in range(2):
                eng.wait_ge(pe_sem, P_LAST_R + 1 + s)
                eng.activation(rt_sb[:, s, :], ps_tp[0:16, 0:128], mybir.ActivationFunctionType.Copy).then_inc(act_sem, 1)
            eng.wait_ge(act_sem, A_RT)
            dst1 = rflat1_sb.ap()[0:1].rearrange("o (t p) -> o t p", p=128)
            eng.dma_start(out=dst1, in_=rt_sb[:, 1, :]).then_inc(dma_sem, 16)


        # ---------------- vector (DVE) engine ----------------
        @block.vector
        def _(eng):
            for it in range(ITERS - 1):
                eng.wait_ge(pe_sem, P_R(it))
                eng.reciprocal(w_sb.ap().rearrange("p t s -> p (t s)"), ps_r[:, 0:32]).then_inc(dve_sem, 1)
                eng.wait_ge(pe_sem, P_C(it))
                eng.tensor_copy(cpart_sb[0:64, :], ps_c[0:64, 0:1])
                eng.tensor_copy(cpart_sb[64:128, :], ps_c[64:128, 1:2]).then_inc(dve_sem, 1)
                eng.wait_ge(dma_sem, 16 * (2 * it + 2))
                eng.tensor_reduce(
                    csum_sb[:], g8_sb[:], mybir.AxisListType.X, mybir.AluOpType.add
                ).then_inc(dve_sem, 1)
                eng.wait_ge(dve_sem, V_RD(it))
                eng.reciprocal(v_sb[0:64, 0:1], csum_sb[0:64, :])
                eng.reciprocal(v_sb[64:128, 1:2], csum_sb[64:128, :]).then_inc(dve_sem, 1)
            eng.wait_ge(pe_sem, P_LAST_R)
            eng.wait_ge(act_sem, A_TEXP)
            eng.tensor_mul(u_sb[:], ps_r[:, 0:32], texp_sb[:]).then_inc(dve_sem, 1)
            eng.wait_ge(dve_sem, V_U)
            eng.scalar_tensor_tensor(
                rlog2_sb.ap().rearrange("p t s -> p (t s)"),
                u_sb[:],
                1.0,
                rlog_sb.ap().rearrange("p t s -> p (t s)"),
                op0=mybir.AluOpType.subtract,
                op1=mybir.AluOpType.add,
            ).then_inc(dve_sem, 1)
            eng.wait_ge(pe_sem, P_ZB)
            eng.wait_ge(dve_sem, V_RL2)
            eng.tensor_sub(zt_sb[:], affT_sb[:], ps_aff[:, 0:TOK]).then_inc(dve_sem, 1)

        # ---------------- gpsimd: collectives ----------------
        @block.gpsimd
        def _(eng):
            for it in range(ITERS - 1):
                eng.wait_ge(dma_sem, 16 * (2 * it + 1))
                eng.collective_compute(
                    "AllGather",
                    mybir.AluOpType.bypass,
                    ins=[cc_in[:]],
                    outs=[cc_out[:]],
                    replica_groups=[core_ids],
                ).then_inc(cc_sem, 1)

    return nc


_CACHE = {}


def _get_nc():
    if "nc" not in _CACHE:
        _CACHE["nc"] = _build_nc()
    return _CACHE["nc"]


def make_in_maps(input_features, expert_centroids):
    feats = np.ascontiguousarray(np.asarray(input_features, dtype=np.float32).reshape(-1, D))
    cent = np.asarray(expert_centroids, dtype=np.float32).reshape(SE, D)

    featsT = np.ascontiguousarray(feats.T)
    centT = np.ascontiguousarray(cent.T)
    ident = np.eye(128, dtype=np.float32)
    ones = np.ones((1, 64), dtype=np.float32)
    onesc = np.ones((128, 1), dtype=np.float32)
    v0 = np.zeros((SE, 2), np.float32)
    v0[0:64, 0] = 1.0
    v0[64:128, 1] = 1.0

    in_maps = []
    for c in range(N_CORES):
        in_maps.append(
            {
                "featsT": np.ascontiguousarray(featsT[:, TOK * c : TOK * (c + 1)]),
                "centT": centT,
                "ident": ident,
                "ones": ones,
                "onesc": onesc,
                "v0": v0,
            }
        )
    return in_maps


def kernel(input_features: np.ndarray, expert_centroids: np.ndarray):
    in_maps = make_in_maps(input_features, expert_centroids)
    nc = _get_nc()
    res = run_bass_kernel_spmd(nc, in_maps, list(range(N_CORES)))

    zt = np.concatenate([res.results[c]["zt"] for c in range(N_CORES)], axis=1)
    afft = np.concatenate([res.results[c]["afft"] for c in range(N_CORES)], axis=1)

    Z = zt.reshape(KSLOT, E, N)
    A = afft.reshape(KSLOT, E, N)
    idx = np.empty((KSLOT, E, CAP), np.int32)
    vals = np.empty((KSLOT, E, CAP), np.float32)
    for k in range(KSLOT):
        for e in range(E):
            col = Z[k, e]
            part = np.sort(np.argpartition(-col, CAP - 1)[:CAP])
            order = part[np.argsort(-col[part], kind="stable")]
            idx[k, e] = order.astype(np.int32)
            vals[k, e] = A[k, e, order]
    return idx, vals



# revision 50
# speedup vs baseline: 5.5589x; 5.5589x over previous
"""Trainium2 Bass kernel for nn_BaseLayerGate (MoE balanced routing).

8 NeuronCores, data-parallel over tokens:
  - Each core owns a 2048-token shard. Affinity matmul aff^T = centT.T @ featsT
    on the tensor engine (fp32r: full fp32 precision at 1 cycle/row).
  - Sinkhorn (10 iters) in reciprocal-potential form:
      R_sum[n]  = sum_se E0[n,se] * V[se]      (PE matvec, V = slot-masked 1/C_sum)
      C_sum[se] = sum_n  E0[n,se] * W[n]       (PE matvec accum, W = 1/R_sum)
    The token-direction sum is global: per-expert partials are allreduced
    across the 8 cores with a 3-round XOR butterfly over remote DMA
    (remote_dma_broadcast with relative dests; descs pre-generated on the
    gpsimd SWDGE ring, fired by trigger_dma). 10 R-steps, 9 C-steps (the
    10th C-step is a uniform per-column shift and cannot change top-k
    ordering).
  - Outputs: aff^T (fp32) and the final R_sum; the host computes
    Z = aff - ln(R) (fp64 ln), does the per-expert top-k and gathers vals.
"""

import numpy as np

import concourse.bass as bass
from concourse import library_config, mybir
from concourse.bass_utils import run_bass_kernel_spmd


def _install_sim_topology_fallback():
    """The instruction simulator resolves remote-DMA routing through libndbg
    driver ioctls; in driverless containers those fail. Install a
    deterministic single-node topology so the simulator can model the
    XOR-relative remote DMAs. Any bijective nc mapping is equivalent for the
    XOR butterfly (on hardware the Q7 resolves relative dests itself, so this
    fallback never affects device execution)."""
    import concourse.libnrt as libnrt

    try:
        libnrt.get_trn2_nc_mapping()
        return
    except Exception:
        pass
    ncmap = {(d, i): i for d in range(16) for i in range(8)}
    ridmap = {d: d for d in range(16)}
    libnrt.get_trn2_nc_mapping = lambda: ncmap
    libnrt.get_device_id_to_routing_id_mapping = lambda: ridmap
    for f in (
        libnrt.nc_to_real_nc,
        libnrt.CoreAddress.from_pnc,
        libnrt._real_nc_to_pnc_mapping,
        libnrt.get_routing_id_to_device_id_mapping,
    ):
        try:
            f.cache_clear()
        except AttributeError:
            pass
    import concourse.bass_interp as bass_interp

    bass_interp.get_device_id_to_routing_id_mapping = (
        libnrt.get_device_id_to_routing_id_mapping
    )


_install_sim_topology_fallback()

N_CORES = 8
N = 16384
D = 1024
KSLOT = 2
E = 64
SE = KSLOT * E
CAP = N // E
TOK = N // N_CORES
ITERS = 10

F32 = mybir.dt.float32
F32R = mybir.dt.float32r

# ---- semaphore schedule ----------------------------------------------------
# pe_sem: aff col-block n done = 1+n (n=0..3) | tp0..tp2 = 5..7 | R(0)=8 |
# tp3=9 | C(k)=10+2k | R(k>=1)=9+2k
def P_B(n):
    return 1 + n
def P_R(k):
    return 8 if k == 0 else 9 + 2 * k
def P_C(k):
    return 10 + 2 * k
P_LAST_R = P_R(ITERS - 1)                      # 27
P_TP = [5, 6, 7, 9]

# act_sem: exp blocks 1..4 | e0tm copies 5..8 | afft copy 9 | rfin copy 10
A_EXP = 4
A_E0TM = 8
A_AFFT = 9
A_RFIN = 10

# dve_sem per iter k (0..8):
# recipW=6k+1, cpart=6k+2, s1=6k+3, s2=6k+4, csum=6k+5, v=6k+6
def V_W(k):
    return 6 * k + 1
def V_CP(k):
    return 6 * k + 2
def V_S1(k):
    return 6 * k + 3
def V_S2(k):
    return 6 * k + 4
def V_CS(k):
    return 6 * k + 5
def V_V(k):
    return 6 * k + 6

# butterfly rounds: (src slot, dst slot, xor distance, rdest slot index)
ROUNDS = [(0, 1, 1), (2, 3, 2), (4, 5, 4)]
PRELOAD = 4   # iterations of preps generated before the trigger loop


def _build_nc():
    nc = bass.Bass()

    featsT_in = nc.declare_dram_parameter("featsT", [D, TOK], F32, isOutput=False)
    centT_in = nc.declare_dram_parameter("centT", [D, SE], F32, isOutput=False)
    v0_in = nc.declare_dram_parameter("v0", [SE, 2], F32, isOutput=False)
    ident_in = nc.declare_dram_parameter("ident", [128, 128], F32, isOutput=False)

    aff_out = nc.declare_dram_parameter("afft", [SE, TOK], F32, isOutput=True)
    rfin_out = nc.declare_dram_parameter("rfin", [128, 32], F32, isOutput=True)

    from contextlib import ExitStack
    es = ExitStack()
    featsT_sb = es.enter_context(nc.sbuf_tensor("featsT_sb", [128, 8, TOK], F32))
    centT_sb = es.enter_context(nc.sbuf_tensor("centT_sb", [128, 8, SE], F32))
    e0t_sb = es.enter_context(nc.sbuf_tensor("e0t_sb", [128, TOK], F32))
    afft_sb = es.enter_context(nc.sbuf_tensor("afft_sb", [128, TOK], F32))
    e0tm_sb = es.enter_context(nc.sbuf_tensor("e0tm_sb", [128, 16, 128], F32))
    ident_sb = es.enter_context(nc.sbuf_tensor("ident_sb", [128, 128], F32))
    v_sb = es.enter_context(nc.sbuf_tensor("v_sb", [128, 2], F32))
    w_sb = es.enter_context(nc.sbuf_tensor("w_sb", [128, 16, 2], F32))
    # exchange slots, one bank per iteration (no reuse -> no WAR hazards):
    # slots: 0=own cpart, 1=recv d1, 2=s1, 3=recv d2, 4=s2, 5=recv d4
    ex_sb = es.enter_context(nc.sbuf_tensor("ex_sb", [128, ITERS - 1, 6], F32))
    csum_sb = es.enter_context(nc.sbuf_tensor("csum_sb", [128, 1], F32))
    rfin_sb = es.enter_context(nc.sbuf_tensor("rfin_sb", [128, 32], F32))
    ps_aff = es.enter_context(nc.psum_tensor("ps_aff", [128, TOK], F32))
    ps_tp = es.enter_context(nc.psum_tensor("ps_tp", [128, 512], F32))
    ps_tp2 = es.enter_context(nc.psum_tensor("ps_tp2", [128, 512], F32))
    ps_r = es.enter_context(nc.psum_tensor("ps_r", [128, 512], F32))
    ps_c = es.enter_context(nc.psum_tensor("ps_c", [128, 512], F32))
    block = es.enter_context(nc.Block())
    csem = es.enter_context(nc.semaphore("csem"))      # centT
    vsem = es.enter_context(nc.semaphore("vsem"))      # v0
    isem = es.enter_context(nc.semaphore("isem"))      # ident
    out_sem = es.enter_context(nc.semaphore("out_sem"))
    fsems = [es.enter_context(nc.semaphore(f"fsem{k}")) for k in range(8)]
    pe_sem = es.enter_context(nc.semaphore("pe_sem"))
    act_sem = es.enter_context(nc.semaphore("act_sem"))
    dve_sem = es.enter_context(nc.semaphore("dve_sem"))
    lsem = es.enter_context(nc.semaphore("lsem"))
    tsem = es.enter_context(nc.semaphore("tsem"))  # trigger completion (normal sem)
    # one remote sem per (round, iteration): single update, single wait
    rsems = [
        [es.enter_context(nc.semaphore(f"rsem{r}_{k}")) for k in range(ITERS - 1)]
        for r in range(3)
    ]

    with es:
        # ---------------- sync engine (SP): DMA queue 1 ----------------
        @block.sync
        def _(eng):
            for k in (0, 3):
                eng.dma_start(
                    out=featsT_sb[:, k, :], in_=featsT_in[128 * k : 128 * (k + 1), :]
                ).then_inc(fsems[k], 16)
            eng.dma_start(out=v_sb[:], in_=v0_in[:]).then_inc(vsem, 16)
            eng.dma_start(
                out=featsT_sb[:, 6, :], in_=featsT_in[128 * 6 : 128 * 7, :]
            ).then_inc(fsems[6], 16)

            eng.wait_ge(act_sem, A_AFFT)
            eng.dma_start(out=aff_out[:], in_=afft_sb[:]).then_inc(out_sem, 16)
            # (afft copy lands on ACT after the e0tm copies, during sinkhorn)
            eng.wait_ge(act_sem, A_RFIN)
            eng.dma_start(out=rfin_out[:], in_=rfin_sb[:]).then_inc(out_sem, 16)
            eng.wait_ge(out_sem, 32)

        # ---------------- vector engine (DVE): sinkhorn elementwise ----
        @block.vector
        def _(eng):
            for k in range(ITERS - 1):
                eng.wait_ge(pe_sem, P_R(k))
                eng.reciprocal(
                    w_sb.ap().rearrange("p t s -> p (t s)"), ps_r[:, 0:32]
                ).then_inc(dve_sem, 1)
                eng.wait_ge(pe_sem, P_C(k))
                eng.tensor_copy(ex_sb[0:64, k, 0:1], ps_c[0:64, 0:1])
                eng.tensor_copy(ex_sb[64:128, k, 0:1], ps_c[64:128, 1:2]).then_inc(
                    dve_sem, 1
                )
                eng.wait_ge(rsems[0][k], 2)
                eng.wait_ge(dve_sem, V_CP(k))
                eng.tensor_add(
                    ex_sb[:, k, 2:3], ex_sb[:, k, 0:1], ex_sb[:, k, 1:2]
                ).then_inc(dve_sem, 1)
                eng.wait_ge(rsems[1][k], 2)
                eng.wait_ge(dve_sem, V_S1(k))
                eng.tensor_add(
                    ex_sb[:, k, 4:5], ex_sb[:, k, 2:3], ex_sb[:, k, 3:4]
                ).then_inc(dve_sem, 1)
                eng.wait_ge(rsems[2][k], 2)
                eng.wait_ge(dve_sem, V_S2(k))
                eng.tensor_add(
                    csum_sb[:], ex_sb[:, k, 4:5], ex_sb[:, k, 5:6]
                ).then_inc(dve_sem, 1)
                eng.wait_ge(dve_sem, V_CS(k))
                eng.reciprocal(v_sb[0:64, 0:1], csum_sb[0:64, :])
                eng.reciprocal(v_sb[64:128, 1:2], csum_sb[64:128, :]).then_inc(
                    dve_sem, 1
                )

        # ---------------- scalar engine (ACT): DMA queue 2 + exp/copies --
        @block.scalar
        def _(eng):
            eng.dma_start(out=ident_sb[:], in_=ident_in[:]).then_inc(isem, 16)
            for k in (1, 4, 7):
                eng.dma_start(
                    out=featsT_sb[:, k, :], in_=featsT_in[128 * k : 128 * (k + 1), :]
                ).then_inc(fsems[k], 16)

            for bkl in range(4):
                eng.wait_ge(pe_sem, P_B(bkl))
                eng.activation(
                    e0t_sb[:, 512 * bkl : 512 * (bkl + 1)],
                    ps_aff[:, 512 * bkl : 512 * (bkl + 1)],
                    mybir.ActivationFunctionType.Exp,
                ).then_inc(act_sem, 1)
            for j in range(4):
                eng.wait_ge(pe_sem, P_TP[j])
                buf = ps_tp if j % 2 == 0 else ps_tp2
                eng.activation(
                    e0tm_sb.ap().rearrange("p t c -> p (t c)")[:, 512 * j : 512 * (j + 1)],
                    buf[:, 0:512],
                    mybir.ActivationFunctionType.Copy,
                ).then_inc(act_sem, 1)
            eng.activation(
                afft_sb[:], ps_aff[:, 0:TOK], mybir.ActivationFunctionType.Copy
            ).then_inc(act_sem, 1)
            eng.wait_ge(pe_sem, P_LAST_R)
            eng.activation(
                rfin_sb[:], ps_r[:, 0:32], mybir.ActivationFunctionType.Copy
            ).then_inc(act_sem, 1)

        # ---------------- tensor engine (PE) ----------------
        @block.tensor
        def _(eng):
            # affinity matmul fp32 (exact): chunks 0-6 k-outer (pipelines with
            # the feats DMAs), then chunk 7 per column block so the exp /
            # transpose chain can chase the matmul tail block by block
            eng.wait_ge(csem, 16)
            for k in range(7):
                eng.wait_ge(fsems[k], 16)
                for n in range(4):
                    eng.matmul(
                        ps_aff[:, 512 * n : 512 * (n + 1)],
                        centT_sb[:, k, :],
                        featsT_sb[:, k, 512 * n : 512 * (n + 1)],
                        start=(k == 0),
                        stop=False,
                    )
            eng.wait_ge(fsems[7], 16)
            for n in range(4):
                eng.matmul(
                    ps_aff[:, 512 * n : 512 * (n + 1)],
                    centT_sb[:, 7, :],
                    featsT_sb[:, 7, 512 * n : 512 * (n + 1)],
                    start=False,
                    stop=True,
                ).then_inc(pe_sem, 1)

            # transposes of e0t into token-major (4 chunks of 128 per block)
            def tp_block(j):
                buf = ps_tp if j % 2 == 0 else ps_tp2
                if j == 0:
                    eng.wait_ge(isem, 16)         # ident loaded
                eng.wait_ge(act_sem, j + 1)       # exp block j done
                if j >= 2:
                    eng.wait_ge(act_sem, 4 + j - 1)  # ACT copied block j-2
                for t in range(4 * j, 4 * j + 4):
                    mm = eng.transpose(
                        buf[:, 128 * (t % 4) : 128 * (t % 4 + 1)],
                        e0t_sb[:, 128 * t : 128 * (t + 1)],
                        ident_sb[:],
                    )
                mm.then_inc(pe_sem, 1)

            for j in range(3):
                tp_block(j)

            # iteration-0 R-step (needs full exp + v0)
            eng.wait_ge(act_sem, A_EXP)
            eng.wait_ge(vsem, 16)
            for t in range(16):
                mm = eng.matmul(
                    ps_r[:, 2 * t : 2 * (t + 1)],
                    e0t_sb[:, 128 * t : 128 * (t + 1)],
                    v_sb[:],
                    start=True,
                    stop=True,
                )
            mm.then_inc(pe_sem, 1)

            tp_block(3)

            for k in range(ITERS):
                if k > 0:
                    eng.wait_ge(dve_sem, V_V(k - 1))
                    for t in range(16):
                        mm = eng.matmul(
                            ps_r[:, 2 * t : 2 * (t + 1)],
                            e0t_sb[:, 128 * t : 128 * (t + 1)],
                            v_sb[:],
                            start=True,
                            stop=True,
                        )
                    mm.then_inc(pe_sem, 1)

                if k < ITERS - 1:
                    if k == 0:
                        eng.wait_ge(act_sem, A_E0TM)
                    eng.wait_ge(dve_sem, V_W(k))
                    for t in range(16):
                        mm = eng.matmul(
                            ps_c[:, 0:2],
                            e0tm_sb[:, t, :],
                            w_sb[:, t, :],
                            start=(t == 0),
                            stop=(t == 15),
                        )
                    mm.then_inc(pe_sem, 1)

        # ---------------- gpsimd (Pool): DMA queue 3 + RDMA butterfly ----
        @block.gpsimd
        def _(eng):
            eng.load_library(library_config.remote_dma)
            src_ap = centT_in.ap().rearrange("(k p) e -> p k e", p=128)
            with nc.allow_non_contiguous_dma(reason="8 strided 512B rows per partition"):
                eng.dma_start(out=centT_sb[:], in_=src_ap).then_inc(csem, 16)
            for k in (2, 5):
                eng.dma_start(
                    out=featsT_sb[:, k, :], in_=featsT_in[128 * k : 128 * (k + 1), :]
                ).then_inc(fsems[k], 16)

            def gen_preps(k):
                for r, (src, dst, delta) in enumerate(ROUNDS):
                    rdests = [None] * 8
                    rdests[delta] = (0, delta)
                    eng.remote_dma_broadcast(
                        out_ap=ex_sb[:, k, dst : dst + 1],
                        in_ap=ex_sb[:, k, src : src + 1],
                        remote_sem=rsems[r][k],
                        local_sem=lsem,
                        rdests=rdests,
                    )

            for k in range(min(PRELOAD, ITERS - 1)):
                gen_preps(k)
            for k in range(ITERS - 1):
                eng.wait_ge(dve_sem, V_CP(k))
                eng.trigger_dma(1).then_inc(tsem, 1)
                eng.wait_ge(dve_sem, V_S1(k))
                eng.trigger_dma(1).then_inc(tsem, 1)
                eng.wait_ge(dve_sem, V_S2(k))
                eng.trigger_dma(1).then_inc(tsem, 1)
                if k + PRELOAD <= ITERS - 2:
                    gen_preps(k + PRELOAD)
            # all sends complete before block drain
            eng.wait_ge(tsem, 3 * (ITERS - 1))

    # raw Bass doesn't run the extended-inst ISA codegen pass; without it the
    # NEFF compiler sees empty .instr ("ISA wrong length")
    from concourse.library_overlay import lower_extended_insts

    lower_extended_insts(nc)
    return nc


_CACHE = {}


def _get_nc():
    if "nc" not in _CACHE:
        _CACHE["nc"] = _build_nc()
    return _CACHE["nc"]


def make_in_maps(input_features, expert_centroids):
    feats = np.ascontiguousarray(
        np.asarray(input_features, dtype=np.float32).reshape(-1, D)
    )
    cent = np.asarray(expert_centroids, dtype=np.float32).reshape(SE, D)

    featsT = np.ascontiguousarray(feats.T)
    centT = np.ascontiguousarray(cent.T)
    ident = np.eye(128, dtype=np.float32)
    v0 = np.zeros((SE, 2), np.float32)
    v0[0:64, 0] = 1.0
    v0[64:128, 1] = 1.0

    in_maps = []
    for c in range(N_CORES):
        in_maps.append(
            {
                "featsT": np.ascontiguousarray(featsT[:, TOK * c : TOK * (c + 1)]),
                "centT": centT,
                "ident": ident,
                "v0": v0,
            }
        )
    return in_maps


def kernel(input_features: np.ndarray, expert_centroids: np.ndarray):
    in_maps = make_in_maps(input_features, expert_centroids)
    nc = _get_nc()
    res = run_bass_kernel_spmd(nc, in_maps, list(range(N_CORES)))

    afft = np.concatenate([res.results[c]["afft"] for c in range(N_CORES)], axis=1)
    # rfin[c][p, 2t+s] = R(token c*2048 + t*128 + p, slot s)
    rfin = np.stack([res.results[c]["rfin"] for c in range(N_CORES)])  # [8,128,32]
    r = rfin.reshape(N_CORES, 128, 16, 2).transpose(0, 2, 1, 3).reshape(N, 2)

    A = afft.reshape(KSLOT, E, N)
    lnr = np.log(r.astype(np.float64)).astype(np.float32)  # [N, 2]
    idx = np.empty((KSLOT, E, CAP), np.int32)
    vals = np.empty((KSLOT, E, CAP), np.float32)
    for k in range(KSLOT):
        Z = A[k] - lnr[:, k][None, :]
        for e in range(E):
            col = Z[e]
            part = np.sort(np.argpartition(-col, CAP - 1)[:CAP])
            order = part[np.argsort(-col[part], kind="stable")]
            idx[k, e] = order.astype(np.int32)
            vals[k, e] = A[k, e, order]
    return idx, vals


# revision 63
# speedup vs baseline: 5.9872x; 1.0771x over previous
"""Trainium2 Bass kernel for nn_BaseLayerGate (MoE balanced routing).

8 NeuronCores, data-parallel over tokens:
  - Each core owns a 2048-token shard. Affinity matmul aff^T = centT.T @ featsT
    on the tensor engine (fp32r: full fp32 precision at 1 cycle/row).
  - Sinkhorn (10 iters) in reciprocal-potential form:
      R_sum[n]  = sum_se E0[n,se] * V[se]      (PE matvec, V = slot-masked 1/C_sum)
      C_sum[se] = sum_n  E0[n,se] * W[n]       (PE matvec accum, W = 1/R_sum)
    The token-direction sum is global: per-expert partials are allreduced
    across the 8 cores with a 3-round XOR butterfly over remote DMA
    (remote_dma_broadcast with relative dests; descs pre-generated on the
    gpsimd SWDGE ring, fired by trigger_dma). 10 R-steps, 9 C-steps (the
    10th C-step is a uniform per-column shift and cannot change top-k
    ordering).
  - Outputs: aff^T (fp32) and the final R_sum; the host computes
    Z = aff - ln(R) (fp64 ln), does the per-expert top-k and gathers vals.
"""

import numpy as np

import concourse.bass as bass
from concourse import library_config, mybir
from concourse.bass_utils import run_bass_kernel_spmd


def _install_sim_topology_fallback():
    """The instruction simulator resolves remote-DMA routing through libndbg
    driver ioctls; in driverless containers those fail. Install a
    deterministic single-node topology so the simulator can model the
    XOR-relative remote DMAs. Any bijective nc mapping is equivalent for the
    XOR butterfly (on hardware the Q7 resolves relative dests itself, so this
    fallback never affects device execution)."""
    import concourse.libnrt as libnrt

    try:
        libnrt.get_trn2_nc_mapping()
        return
    except Exception:
        pass
    ncmap = {(d, i): i for d in range(16) for i in range(8)}
    ridmap = {d: d for d in range(16)}
    libnrt.get_trn2_nc_mapping = lambda: ncmap
    libnrt.get_device_id_to_routing_id_mapping = lambda: ridmap
    for f in (
        libnrt.nc_to_real_nc,
        libnrt.CoreAddress.from_pnc,
        libnrt._real_nc_to_pnc_mapping,
        libnrt.get_routing_id_to_device_id_mapping,
    ):
        try:
            f.cache_clear()
        except AttributeError:
            pass
    import concourse.bass_interp as bass_interp

    bass_interp.get_device_id_to_routing_id_mapping = (
        libnrt.get_device_id_to_routing_id_mapping
    )


_install_sim_topology_fallback()

N_CORES = 8
N = 16384
D = 1024
KSLOT = 2
E = 64
SE = KSLOT * E
CAP = N // E
TOK = N // N_CORES
ITERS = 10

F32 = mybir.dt.float32
F32R = mybir.dt.float32r

# ---- semaphore schedule ----------------------------------------------------
# pe_sem: aff col-block n done = 1+n (n=0..3) | tp0=5 | tp1=6 | R(0)=7 |
# tp2=8 | tp3=9 | C(k)=10+2k | R(k>=1)=9+2k
def P_B(n):
    return 1 + n
def P_R(k):
    return 7 if k == 0 else 9 + 2 * k
def P_C(k):
    return 10 + 2 * k
P_LAST_R = P_R(ITERS - 1)                      # 27
P_TP = [5, 6, 8, 9]

# act_sem: exp blocks 1..4 | c0=5 | c2=6 | afft copy 7 | rfin copy 8
A_EXP = 4
A_AFFT = 7
A_RFIN = 8
# cpsem (DVE copies): c1=1 | c3=2

# dve_sem per iter k (0..8):
# recipW=6k+1, cpart=6k+2, s1=6k+3, s2=6k+4, csum=6k+5, v=6k+6
def V_W(k):
    return 6 * k + 1
def V_CP(k):
    return 6 * k + 2
def V_S1(k):
    return 6 * k + 3
def V_S2(k):
    return 6 * k + 4
def V_CS(k):
    return 6 * k + 5
def V_V(k):
    return 6 * k + 6

# butterfly rounds: (src slot, dst slot, xor distance, rdest slot index)
ROUNDS = [(0, 1, 1), (2, 3, 2), (4, 5, 4)]
PRELOAD = 4   # iterations of preps generated before the trigger loop


def _build_nc():
    nc = bass.Bass()

    featsT_in = nc.declare_dram_parameter("featsT", [D, TOK], F32, isOutput=False)
    centT_in = nc.declare_dram_parameter("centT", [D, SE], F32, isOutput=False)
    v0_in = nc.declare_dram_parameter("v0", [SE, 2], F32, isOutput=False)
    ident_in = nc.declare_dram_parameter("ident", [128, 128], F32, isOutput=False)

    aff_out = nc.declare_dram_parameter("afft", [SE, TOK], F32, isOutput=True)
    rfin_out = nc.declare_dram_parameter("rfin", [128, 32], F32, isOutput=True)

    from contextlib import ExitStack
    es = ExitStack()
    featsT_sb = es.enter_context(nc.sbuf_tensor("featsT_sb", [128, 8, TOK], F32))
    centT_sb = es.enter_context(nc.sbuf_tensor("centT_sb", [128, 8, SE], F32))
    e0t_sb = es.enter_context(nc.sbuf_tensor("e0t_sb", [128, TOK], F32))
    afft_sb = es.enter_context(nc.sbuf_tensor("afft_sb", [128, TOK], F32))
    e0tm_sb = es.enter_context(nc.sbuf_tensor("e0tm_sb", [128, 16, 128], F32))
    ident_sb = es.enter_context(nc.sbuf_tensor("ident_sb", [128, 128], F32))
    escr_sb = es.enter_context(nc.sbuf_tensor("escr_sb", [128, 1], F32))
    v_sb = es.enter_context(nc.sbuf_tensor("v_sb", [128, 2], F32))
    w_sb = es.enter_context(nc.sbuf_tensor("w_sb", [128, 16, 2], F32))
    # exchange slots, one bank per iteration (no reuse -> no WAR hazards):
    # slots: 0=own cpart, 1=recv d1, 2=s1, 3=recv d2, 4=s2, 5=recv d4
    ex_sb = es.enter_context(nc.sbuf_tensor("ex_sb", [128, ITERS - 1, 6], F32))
    csum_sb = es.enter_context(nc.sbuf_tensor("csum_sb", [128, 1], F32))
    rfin_sb = es.enter_context(nc.sbuf_tensor("rfin_sb", [128, 32], F32))
    ps_aff = es.enter_context(nc.psum_tensor("ps_aff", [128, TOK], F32))
    ps_tp = es.enter_context(nc.psum_tensor("ps_tp", [128, 512], F32))
    ps_tp2 = es.enter_context(nc.psum_tensor("ps_tp2", [128, 512], F32))
    ps_r = es.enter_context(nc.psum_tensor("ps_r", [128, 512], F32))
    ps_c = es.enter_context(nc.psum_tensor("ps_c", [128, 512], F32))
    block = es.enter_context(nc.Block())
    csem = es.enter_context(nc.semaphore("csem"))      # centT
    vsem = es.enter_context(nc.semaphore("vsem"))      # v0
    isem = es.enter_context(nc.semaphore("isem"))      # ident
    out_sem = es.enter_context(nc.semaphore("out_sem"))
    fsems = [es.enter_context(nc.semaphore(f"fsem{k}")) for k in range(8)]
    f0a_sem = es.enter_context(nc.semaphore("f0a_sem"))
    pe_sem = es.enter_context(nc.semaphore("pe_sem"))
    act_sem = es.enter_context(nc.semaphore("act_sem"))
    dve_sem = es.enter_context(nc.semaphore("dve_sem"))
    lsem = es.enter_context(nc.semaphore("lsem"))
    tsem = es.enter_context(nc.semaphore("tsem"))  # trigger completion (normal sem)
    # one remote sem per (round, iteration): single update, single wait
    rsems = [
        [es.enter_context(nc.semaphore(f"rsem{r}_{k}")) for k in range(ITERS - 1)]
        for r in range(3)
    ]
    cpsem = es.enter_context(nc.semaphore("cpsem"))

    with es:
        # ---------------- sync engine (SP): DMA queue 1 ----------------
        @block.sync
        def _(eng):
            # chunk 0 split into token halves so the matmul starts ~1.6us
            # earlier (column block n covers tokens 512n..512n+511)
            eng.dma_start(
                out=featsT_sb[:, 0, 0:1024], in_=featsT_in[0:128, 0:1024]
            ).then_inc(f0a_sem, 16)
            eng.dma_start(
                out=featsT_sb[:, 0, 1024:2048], in_=featsT_in[0:128, 1024:2048]
            ).then_inc(fsems[0], 16)
            eng.dma_start(
                out=featsT_sb[:, 3, :], in_=featsT_in[128 * 3 : 128 * 4, :]
            ).then_inc(fsems[3], 16)
            eng.dma_start(out=v_sb[:], in_=v0_in[:]).then_inc(vsem, 16)
            eng.dma_start(
                out=featsT_sb[:, 6, :], in_=featsT_in[128 * 6 : 128 * 7, :]
            ).then_inc(fsems[6], 16)

            eng.wait_ge(act_sem, A_AFFT)
            eng.dma_start(out=aff_out[:], in_=afft_sb[:]).then_inc(out_sem, 16)
            eng.wait_ge(act_sem, A_RFIN)
            eng.dma_start(out=rfin_out[:], in_=rfin_sb[:]).then_inc(out_sem, 16)
            eng.wait_ge(out_sem, 32)

        # ---------------- vector engine (DVE): e0tm copies + sinkhorn ----
        @block.vector
        def _(eng):
            eng.wait_ge(pe_sem, P_TP[1])
            eng.tensor_copy(
                e0tm_sb.ap().rearrange("p t c -> p (t c)")[:, 512:1024],
                ps_tp2[:, 0:512],
            ).then_inc(cpsem, 1)
            eng.wait_ge(pe_sem, P_TP[3])
            eng.tensor_copy(
                e0tm_sb.ap().rearrange("p t c -> p (t c)")[:, 1536:2048],
                ps_tp2[:, 0:512],
            ).then_inc(cpsem, 1)
            for k in range(ITERS - 1):
                eng.wait_ge(pe_sem, P_R(k))
                eng.reciprocal(
                    w_sb.ap().rearrange("p t s -> p (t s)"), ps_r[:, 0:32]
                ).then_inc(dve_sem, 1)
                eng.wait_ge(pe_sem, P_C(k))
                eng.tensor_copy(ex_sb[0:64, k, 0:1], ps_c[0:64, 0:1])
                eng.tensor_copy(ex_sb[64:128, k, 0:1], ps_c[64:128, 1:2]).then_inc(
                    dve_sem, 1
                )
                eng.wait_ge(rsems[0][k], 2)
                eng.wait_ge(dve_sem, V_CP(k))
                eng.tensor_add(
                    ex_sb[:, k, 2:3], ex_sb[:, k, 0:1], ex_sb[:, k, 1:2]
                ).then_inc(dve_sem, 1)
                eng.wait_ge(rsems[1][k], 2)
                eng.wait_ge(dve_sem, V_S1(k))
                eng.tensor_add(
                    ex_sb[:, k, 4:5], ex_sb[:, k, 2:3], ex_sb[:, k, 3:4]
                ).then_inc(dve_sem, 1)
                eng.wait_ge(rsems[2][k], 2)
                eng.wait_ge(dve_sem, V_S2(k))
                eng.tensor_add(
                    csum_sb[:], ex_sb[:, k, 4:5], ex_sb[:, k, 5:6]
                ).then_inc(dve_sem, 1)
                eng.wait_ge(dve_sem, V_CS(k))
                eng.reciprocal(v_sb[0:64, 0:1], csum_sb[0:64, :])
                eng.reciprocal(v_sb[64:128, 1:2], csum_sb[64:128, :]).then_inc(
                    dve_sem, 1
                )

        # ---------------- scalar engine (ACT): DMA queue 2 + exp/copies --
        @block.scalar
        def _(eng):
            eng.dma_start(out=ident_sb[:], in_=ident_in[:]).then_inc(isem, 16)
            eng.dma_start(
                out=featsT_sb[:, 1, :], in_=featsT_in[128 * 1 : 128 * 2, :]
            ).then_inc(fsems[1], 16)
            # preload the Exp activation table while the matmul runs
            eng.activation(
                escr_sb[:], ident_sb[:, 0:1], mybir.ActivationFunctionType.Exp
            )
            for k in (4, 7):
                eng.dma_start(
                    out=featsT_sb[:, k, :], in_=featsT_in[128 * k : 128 * (k + 1), :]
                ).then_inc(fsems[k], 16)

            for bkl in range(4):
                eng.wait_ge(pe_sem, P_B(bkl))
                eng.activation(
                    e0t_sb[:, 512 * bkl : 512 * (bkl + 1)],
                    ps_aff[:, 512 * bkl : 512 * (bkl + 1)],
                    mybir.ActivationFunctionType.Exp,
                ).then_inc(act_sem, 1)
            for j in (0, 2):
                eng.wait_ge(pe_sem, P_TP[j])
                eng.activation(
                    e0tm_sb.ap().rearrange("p t c -> p (t c)")[:, 512 * j : 512 * (j + 1)],
                    ps_tp[:, 0:512],
                    mybir.ActivationFunctionType.Copy,
                ).then_inc(act_sem, 1)
            eng.activation(
                afft_sb[:], ps_aff[:, 0:TOK], mybir.ActivationFunctionType.Copy
            ).then_inc(act_sem, 1)
            eng.wait_ge(pe_sem, P_LAST_R)
            eng.activation(
                rfin_sb[:], ps_r[:, 0:32], mybir.ActivationFunctionType.Copy
            ).then_inc(act_sem, 1)

        # ---------------- tensor engine (PE) ----------------
        @block.tensor
        def _(eng):
            # warm the PE p-state ramp on ident transposes while chunk 0 loads
            # (targets: unused psum regions, one write each)
            eng.wait_ge(isem, 16)
            for wbuf, lo in ((ps_r, 128), (ps_r, 256), (ps_r, 384), (ps_c, 384)):
                eng.transpose(wbuf[:, lo : lo + 128], ident_sb[:], ident_sb[:])

            # affinity matmul fp32 (exact): chunk 0 in token halves (starts as
            # soon as the first half lands), chunks 1-6 k-outer (pipelines
            # with the feats DMAs), then chunk 7 per column block so the
            # exp / transpose chain can chase the matmul tail block by block
            eng.wait_ge(csem, 16)
            eng.wait_ge(f0a_sem, 16)
            for n in (0, 1):
                eng.matmul(
                    ps_aff[:, 512 * n : 512 * (n + 1)],
                    centT_sb[:, 0, :],
                    featsT_sb[:, 0, 512 * n : 512 * (n + 1)],
                    start=True,
                    stop=False,
                )
            eng.wait_ge(fsems[0], 16)
            for n in (2, 3):
                eng.matmul(
                    ps_aff[:, 512 * n : 512 * (n + 1)],
                    centT_sb[:, 0, :],
                    featsT_sb[:, 0, 512 * n : 512 * (n + 1)],
                    start=True,
                    stop=False,
                )
            for k in range(1, 7):
                eng.wait_ge(fsems[k], 16)
                for n in range(4):
                    eng.matmul(
                        ps_aff[:, 512 * n : 512 * (n + 1)],
                        centT_sb[:, k, :],
                        featsT_sb[:, k, 512 * n : 512 * (n + 1)],
                        start=False,
                        stop=False,
                    )
            eng.wait_ge(fsems[7], 16)
            for n in range(4):
                eng.matmul(
                    ps_aff[:, 512 * n : 512 * (n + 1)],
                    centT_sb[:, 7, :],
                    featsT_sb[:, 7, 512 * n : 512 * (n + 1)],
                    start=False,
                    stop=True,
                ).then_inc(pe_sem, 1)

            # transposes of e0t into token-major (4 chunks of 128 per block)
            def tp_block(j):
                buf = ps_tp if j % 2 == 0 else ps_tp2
                eng.wait_ge(act_sem, j + 1)       # exp block j done
                if j == 2:
                    eng.wait_ge(act_sem, 5)       # c0 freed ps_tp
                if j == 3:
                    eng.wait_ge(cpsem, 1)         # c1 freed ps_tp2
                for t in range(4 * j, 4 * j + 4):
                    mm = eng.transpose(
                        buf[:, 128 * (t % 4) : 128 * (t % 4 + 1)],
                        e0t_sb[:, 128 * t : 128 * (t + 1)],
                        ident_sb[:],
                    )
                mm.then_inc(pe_sem, 1)

            tp_block(0)
            tp_block(1)

            # iteration-0 R-step (needs full exp + v0)
            eng.wait_ge(act_sem, A_EXP)
            eng.wait_ge(vsem, 16)
            for t in range(16):
                mm = eng.matmul(
                    ps_r[:, 2 * t : 2 * (t + 1)],
                    e0t_sb[:, 128 * t : 128 * (t + 1)],
                    v_sb[:],
                    start=True,
                    stop=True,
                )
            mm.then_inc(pe_sem, 1)

            tp_block(2)
            tp_block(3)

            for k in range(ITERS):
                if k > 0:
                    eng.wait_ge(dve_sem, V_V(k - 1))
                    for t in range(16):
                        mm = eng.matmul(
                            ps_r[:, 2 * t : 2 * (t + 1)],
                            e0t_sb[:, 128 * t : 128 * (t + 1)],
                            v_sb[:],
                            start=True,
                            stop=True,
                        )
                    mm.then_inc(pe_sem, 1)

                if k < ITERS - 1:
                    if k == 0:
                        eng.wait_ge(act_sem, 6)   # c0 + c2 landed
                        eng.wait_ge(cpsem, 2)     # c1 + c3 landed
                    eng.wait_ge(dve_sem, V_W(k))
                    for t in range(16):
                        mm = eng.matmul(
                            ps_c[:, 0:2],
                            e0tm_sb[:, t, :],
                            w_sb[:, t, :],
                            start=(t == 0),
                            stop=(t == 15),
                        )
                    mm.then_inc(pe_sem, 1)

        # ---------------- gpsimd (Pool): DMA queue 3 + RDMA butterfly ----
        @block.gpsimd
        def _(eng):
            eng.load_library(library_config.remote_dma)
            src_ap = centT_in.ap().rearrange("(k p) e -> p k e", p=128)
            with nc.allow_non_contiguous_dma(reason="8 strided 512B rows per partition"):
                eng.dma_start(out=centT_sb[:], in_=src_ap).then_inc(csem, 16)
            for k in (2, 5):
                eng.dma_start(
                    out=featsT_sb[:, k, :], in_=featsT_in[128 * k : 128 * (k + 1), :]
                ).then_inc(fsems[k], 16)

            def gen_preps(k):
                for r, (src, dst, delta) in enumerate(ROUNDS):
                    rdests = [None] * 8
                    rdests[delta] = (0, delta)
                    eng.remote_dma_broadcast(
                        out_ap=ex_sb[:, k, dst : dst + 1],
                        in_ap=ex_sb[:, k, src : src + 1],
                        remote_sem=rsems[r][k],
                        local_sem=lsem,
                        rdests=rdests,
                    )

            for k in range(min(PRELOAD, ITERS - 1)):
                gen_preps(k)
            for k in range(ITERS - 1):
                eng.wait_ge(dve_sem, V_CP(k))
                eng.trigger_dma(1).then_inc(tsem, 1)
                eng.wait_ge(dve_sem, V_S1(k))
                eng.trigger_dma(1).then_inc(tsem, 1)
                eng.wait_ge(dve_sem, V_S2(k))
                eng.trigger_dma(1).then_inc(tsem, 1)
                if k + PRELOAD <= ITERS - 2:
                    gen_preps(k + PRELOAD)
            # all sends complete before block drain
            eng.wait_ge(tsem, 3 * (ITERS - 1))

    # raw Bass doesn't run the extended-inst ISA codegen pass; without it the
    # NEFF compiler sees empty .instr ("ISA wrong length")
    from concourse.library_overlay import lower_extended_insts

    lower_extended_insts(nc)
    return nc


_CACHE = {}


def _get_nc():
    if "nc" not in _CACHE:
        _CACHE["nc"] = _build_nc()
    return _CACHE["nc"]


def make_in_maps(input_features, expert_centroids):
    feats = np.ascontiguousarray(
        np.asarray(input_features, dtype=np.float32).reshape(-1, D)
    )
    cent = np.asarray(expert_centroids, dtype=np.float32).reshape(SE, D)

    featsT = np.ascontiguousarray(feats.T)
    centT = np.ascontiguousarray(cent.T)
    ident = np.eye(128, dtype=np.float32)
    v0 = np.zeros((SE, 2), np.float32)
    v0[0:64, 0] = 1.0
    v0[64:128, 1] = 1.0

    in_maps = []
    for c in range(N_CORES):
        in_maps.append(
            {
                "featsT": np.ascontiguousarray(featsT[:, TOK * c : TOK * (c + 1)]),
                "centT": centT,
                "ident": ident,
                "v0": v0,
            }
        )
    return in_maps


def kernel(input_features: np.ndarray, expert_centroids: np.ndarray):
    in_maps = make_in_maps(input_features, expert_centroids)
    nc = _get_nc()
    res = run_bass_kernel_spmd(nc, in_maps, list(range(N_CORES)))

    afft = np.concatenate([res.results[c]["afft"] for c in range(N_CORES)], axis=1)
    # rfin[c][p, 2t+s] = R(token c*2048 + t*128 + p, slot s)
    rfin = np.stack([res.results[c]["rfin"] for c in range(N_CORES)])  # [8,128,32]
    r = rfin.reshape(N_CORES, 128, 16, 2).transpose(0, 2, 1, 3).reshape(N, 2)

    A = afft.reshape(KSLOT, E, N)
    lnr = np.log(r.astype(np.float64)).astype(np.float32)  # [N, 2]
    idx = np.empty((KSLOT, E, CAP), np.int32)
    vals = np.empty((KSLOT, E, CAP), np.float32)
    for k in range(KSLOT):
        Z = A[k] - lnr[:, k][None, :]
        for e in range(E):
            col = Z[e]
            part = np.sort(np.argpartition(-col, CAP - 1)[:CAP])
            order = part[np.argsort(-col[part], kind="stable")]
            idx[k, e] = order.astype(np.int32)
            vals[k, e] = A[k, e, order]
    return idx, vals


# revision 70
# speedup vs baseline: 6.1003x; 1.0189x over previous
"""Trainium2 Bass kernel for nn_BaseLayerGate (MoE balanced routing).

8 NeuronCores, data-parallel over tokens:
  - Each core owns a 2048-token shard. Affinity matmul aff^T = centT.T @ featsT
    on the tensor engine (fp32r: full fp32 precision at 1 cycle/row).
  - Sinkhorn (10 iters) in reciprocal-potential form:
      R_sum[n]  = sum_se E0[n,se] * V[se]      (PE matvec, V = slot-masked 1/C_sum)
      C_sum[se] = sum_n  E0[n,se] * W[n]       (PE matvec accum, W = 1/R_sum)
    The token-direction sum is global: per-expert partials are allreduced
    across the 8 cores with a 3-round XOR butterfly over remote DMA
    (remote_dma_broadcast with relative dests; descs pre-generated on the
    gpsimd SWDGE ring, fired by trigger_dma). 10 R-steps, 9 C-steps (the
    10th C-step is a uniform per-column shift and cannot change top-k
    ordering).
  - Outputs: aff^T (fp32) and the final R_sum; the host computes
    Z = aff - ln(R) (fp64 ln), does the per-expert top-k and gathers vals.
"""

import numpy as np

import concourse.bass as bass
from concourse import library_config, mybir
from concourse.bass_utils import run_bass_kernel_spmd


def _install_sim_topology_fallback():
    """The instruction simulator resolves remote-DMA routing through libndbg
    driver ioctls; in driverless containers those fail. Install a
    deterministic single-node topology so the simulator can model the
    XOR-relative remote DMAs. Any bijective nc mapping is equivalent for the
    XOR butterfly (on hardware the Q7 resolves relative dests itself, so this
    fallback never affects device execution)."""
    import concourse.libnrt as libnrt

    try:
        libnrt.get_trn2_nc_mapping()
        return
    except Exception:
        pass
    ncmap = {(d, i): i for d in range(16) for i in range(8)}
    ridmap = {d: d for d in range(16)}
    libnrt.get_trn2_nc_mapping = lambda: ncmap
    libnrt.get_device_id_to_routing_id_mapping = lambda: ridmap
    for f in (
        libnrt.nc_to_real_nc,
        libnrt.CoreAddress.from_pnc,
        libnrt._real_nc_to_pnc_mapping,
        libnrt.get_routing_id_to_device_id_mapping,
    ):
        try:
            f.cache_clear()
        except AttributeError:
            pass
    import concourse.bass_interp as bass_interp

    bass_interp.get_device_id_to_routing_id_mapping = (
        libnrt.get_device_id_to_routing_id_mapping
    )


_install_sim_topology_fallback()

N_CORES = 8
N = 16384
D = 1024
KSLOT = 2
E = 64
SE = KSLOT * E
CAP = N // E
TOK = N // N_CORES
ITERS = 10

F32 = mybir.dt.float32
F32R = mybir.dt.float32r

# ---- semaphore schedule ----------------------------------------------------
# pe_sem: aff col-block n done = 1+n (n=0..3) | tp0=5 | tp1=6 | R(0)=7 |
# tp3=8 | tp2=9 | C(k)=10+2k | R(k>=1)=9+2k
def P_B(n):
    return 1 + n
def P_R(k):
    return 7 if k == 0 else 9 + 2 * k
def P_C(k):
    return 10 + 2 * k
P_LAST_R = P_R(ITERS - 1)                      # 27
P_TP = [5, 6, 9, 8]  # indexed by block: tp2 runs after tp3 on PE

# act_sem: exp blocks 1..4 | c0=5 | c2=6 | afft copy 7 | rfin copy 8
A_EXP = 4
A_AFFT = 7
A_RFIN = 8
# cpsem (DVE copies): c1=1 | c3=2

# dve_sem per iter k (0..8):
# recipW=6k+1, cpart=6k+2, s1=6k+3, s2=6k+4, csum=6k+5, v=6k+6
def V_W(k):
    return 6 * k + 1
def V_CP(k):
    return 6 * k + 2
def V_S1(k):
    return 6 * k + 3
def V_S2(k):
    return 6 * k + 4
def V_CS(k):
    return 6 * k + 5
def V_V(k):
    return 6 * k + 6

# butterfly rounds: (src slot, dst slot, xor distance, rdest slot index)
ROUNDS = [(0, 1, 1), (2, 3, 2), (4, 5, 4)]
PRELOAD = 4   # iterations of preps generated before the trigger loop


def _build_nc():
    nc = bass.Bass()

    featsT_in = nc.declare_dram_parameter("featsT", [D, TOK], F32, isOutput=False)
    centT_in = nc.declare_dram_parameter("centT", [D, SE], F32, isOutput=False)
    v0_in = nc.declare_dram_parameter("v0", [SE, 2], F32, isOutput=False)
    ident_in = nc.declare_dram_parameter("ident", [128, 128], F32, isOutput=False)

    aff_out = nc.declare_dram_parameter("afft", [SE, TOK], F32, isOutput=True)
    rfin_out = nc.declare_dram_parameter("rfin", [128, 32], F32, isOutput=True)

    from contextlib import ExitStack
    es = ExitStack()
    featsT_sb = es.enter_context(nc.sbuf_tensor("featsT_sb", [128, 8, TOK], F32))
    centT_sb = es.enter_context(nc.sbuf_tensor("centT_sb", [128, 8, SE], F32))
    e0t_sb = es.enter_context(nc.sbuf_tensor("e0t_sb", [128, TOK], F32))
    afft_sb = es.enter_context(nc.sbuf_tensor("afft_sb", [128, TOK], F32))
    e0tm_sb = es.enter_context(nc.sbuf_tensor("e0tm_sb", [128, 16, 128], F32))
    ident_sb = es.enter_context(nc.sbuf_tensor("ident_sb", [128, 128], F32))
    escr_sb = es.enter_context(nc.sbuf_tensor("escr_sb", [128, 1], F32))
    v_sb = es.enter_context(nc.sbuf_tensor("v_sb", [128, 2], F32))
    w_sb = es.enter_context(nc.sbuf_tensor("w_sb", [128, 16, 2], F32))
    # exchange slots, one bank per iteration (no reuse -> no WAR hazards):
    # slots: 0=own cpart, 1=recv d1, 2=s1, 3=recv d2, 4=s2, 5=recv d4
    ex_sb = es.enter_context(nc.sbuf_tensor("ex_sb", [128, ITERS - 1, 6], F32))
    csum_sb = es.enter_context(nc.sbuf_tensor("csum_sb", [128, 1], F32))
    rfin_sb = es.enter_context(nc.sbuf_tensor("rfin_sb", [128, 32], F32))
    ps_aff = es.enter_context(nc.psum_tensor("ps_aff", [128, TOK], F32))
    ps_tp = es.enter_context(nc.psum_tensor("ps_tp", [128, 512], F32))
    ps_tp2 = es.enter_context(nc.psum_tensor("ps_tp2", [128, 512], F32))
    ps_r = es.enter_context(nc.psum_tensor("ps_r", [128, 512], F32))
    ps_c = es.enter_context(nc.psum_tensor("ps_c", [128, 512], F32))
    block = es.enter_context(nc.Block())
    csem = es.enter_context(nc.semaphore("csem"))      # centT
    vsem = es.enter_context(nc.semaphore("vsem"))      # v0
    isem = es.enter_context(nc.semaphore("isem"))      # ident
    out_sem = es.enter_context(nc.semaphore("out_sem"))
    fsems = [es.enter_context(nc.semaphore(f"fsem{k}")) for k in range(8)]
    f0a_sem = es.enter_context(nc.semaphore("f0a_sem"))
    pe_sem = es.enter_context(nc.semaphore("pe_sem"))
    act_sem = es.enter_context(nc.semaphore("act_sem"))
    dve_sem = es.enter_context(nc.semaphore("dve_sem"))
    lsem = es.enter_context(nc.semaphore("lsem"))
    tsem = es.enter_context(nc.semaphore("tsem"))  # trigger completion (normal sem)
    # one remote sem per (round, iteration): single update, single wait
    rsems = [
        [es.enter_context(nc.semaphore(f"rsem{r}_{k}")) for k in range(ITERS - 1)]
        for r in range(3)
    ]
    cpsem = es.enter_context(nc.semaphore("cpsem"))

    with es:
        # ---------------- sync engine (SP): DMA queue 1 ----------------
        @block.sync
        def _(eng):
            # chunk 0 split into token halves so the matmul starts ~1.6us
            # earlier (column block n covers tokens 512n..512n+511)
            eng.dma_start(
                out=featsT_sb[:, 0, 0:1024], in_=featsT_in[0:128, 0:1024]
            ).then_inc(f0a_sem, 16)
            eng.dma_start(
                out=featsT_sb[:, 0, 1024:2048], in_=featsT_in[0:128, 1024:2048]
            ).then_inc(fsems[0], 16)
            eng.dma_start(
                out=featsT_sb[:, 3, :], in_=featsT_in[128 * 3 : 128 * 4, :]
            ).then_inc(fsems[3], 16)
            eng.dma_start(out=v_sb[:], in_=v0_in[:]).then_inc(vsem, 16)
            eng.dma_start(
                out=featsT_sb[:, 6, :], in_=featsT_in[128 * 6 : 128 * 7, :]
            ).then_inc(fsems[6], 16)

            eng.wait_ge(act_sem, A_AFFT)
            eng.dma_start(out=aff_out[:], in_=afft_sb[:]).then_inc(out_sem, 16)
            eng.wait_ge(out_sem, 32)

        # ---------------- vector engine (DVE): e0tm copies + sinkhorn ----
        @block.vector
        def _(eng):
            eng.wait_ge(pe_sem, P_TP[1])
            eng.tensor_copy(
                e0tm_sb.ap().rearrange("p t c -> p (t c)")[:, 512:1024],
                ps_tp2[:, 0:512],
            ).then_inc(cpsem, 1)
            eng.wait_ge(pe_sem, P_TP[3])
            eng.tensor_copy(
                e0tm_sb.ap().rearrange("p t c -> p (t c)")[:, 1536:2048],
                ps_c[:, 0:512],
            ).then_inc(cpsem, 1)
            for k in range(ITERS - 1):
                eng.wait_ge(pe_sem, P_R(k))
                eng.reciprocal(
                    w_sb.ap().rearrange("p t s -> p (t s)"), ps_r[:, 0:32]
                ).then_inc(dve_sem, 1)
                eng.wait_ge(pe_sem, P_C(k))
                eng.tensor_copy(ex_sb[0:64, k, 0:1], ps_c[0:64, 0:1])
                eng.tensor_copy(ex_sb[64:128, k, 0:1], ps_c[64:128, 1:2]).then_inc(
                    dve_sem, 1
                )
                eng.wait_ge(rsems[0][k], 2)
                eng.wait_ge(dve_sem, V_CP(k))
                eng.tensor_add(
                    ex_sb[:, k, 2:3], ex_sb[:, k, 0:1], ex_sb[:, k, 1:2]
                ).then_inc(dve_sem, 1)
                eng.wait_ge(rsems[1][k], 2)
                eng.wait_ge(dve_sem, V_S1(k))
                eng.tensor_add(
                    ex_sb[:, k, 4:5], ex_sb[:, k, 2:3], ex_sb[:, k, 3:4]
                ).then_inc(dve_sem, 1)
                eng.wait_ge(rsems[2][k], 2)
                eng.wait_ge(dve_sem, V_S2(k))
                eng.tensor_add(
                    csum_sb[:], ex_sb[:, k, 4:5], ex_sb[:, k, 5:6]
                ).then_inc(dve_sem, 1)
                eng.wait_ge(dve_sem, V_CS(k))
                eng.reciprocal(v_sb[0:64, 0:1], csum_sb[0:64, :])
                eng.reciprocal(v_sb[64:128, 1:2], csum_sb[64:128, :]).then_inc(
                    dve_sem, 1
                )

        # ---------------- scalar engine (ACT): DMA queue 2 + exp/copies --
        @block.scalar
        def _(eng):
            eng.dma_start(out=ident_sb[:], in_=ident_in[:]).then_inc(isem, 16)
            eng.dma_start(
                out=featsT_sb[:, 1, :], in_=featsT_in[128 * 1 : 128 * 2, :]
            ).then_inc(fsems[1], 16)
            # preload the Exp activation table while the matmul runs
            eng.activation(
                escr_sb[:], ident_sb[:, 0:1], mybir.ActivationFunctionType.Exp
            )
            for k in (4, 7):
                eng.dma_start(
                    out=featsT_sb[:, k, :], in_=featsT_in[128 * k : 128 * (k + 1), :]
                ).then_inc(fsems[k], 16)

            for bkl in range(4):
                eng.wait_ge(pe_sem, P_B(bkl))
                eng.activation(
                    e0t_sb[:, 512 * bkl : 512 * (bkl + 1)],
                    ps_aff[:, 512 * bkl : 512 * (bkl + 1)],
                    mybir.ActivationFunctionType.Exp,
                ).then_inc(act_sem, 1)
            for j in (0, 2):
                eng.wait_ge(pe_sem, P_TP[j])
                eng.activation(
                    e0tm_sb.ap().rearrange("p t c -> p (t c)")[:, 512 * j : 512 * (j + 1)],
                    ps_tp[:, 0:512],
                    mybir.ActivationFunctionType.Copy,
                ).then_inc(act_sem, 1)
            eng.activation(
                afft_sb[:], ps_aff[:, 0:TOK], mybir.ActivationFunctionType.Copy
            ).then_inc(act_sem, 1)
            eng.wait_ge(pe_sem, P_LAST_R)
            eng.activation(
                rfin_sb[:], ps_r[:, 0:32], mybir.ActivationFunctionType.Copy
            ).then_inc(act_sem, 1)
            eng.wait_ge(act_sem, A_RFIN)
            eng.dma_start(out=rfin_out[:], in_=rfin_sb[:]).then_inc(out_sem, 16)

        # ---------------- tensor engine (PE) ----------------
        @block.tensor
        def _(eng):
            # warm the PE p-state ramp on ident transposes while chunk 0 loads
            # (targets: unused psum regions, one write each)
            eng.wait_ge(isem, 16)
            for wbuf, lo in ((ps_r, 128), (ps_r, 256), (ps_r, 384)):
                eng.transpose(wbuf[:, lo : lo + 128], ident_sb[:], ident_sb[:])

            # affinity matmul fp32 (exact): chunk 0 in token halves (starts as
            # soon as the first half lands), chunks 1-6 k-outer (pipelines
            # with the feats DMAs), then chunk 7 per column block so the
            # exp / transpose chain can chase the matmul tail block by block
            eng.wait_ge(csem, 16)
            eng.wait_ge(f0a_sem, 16)
            for n in (0, 1):
                eng.matmul(
                    ps_aff[:, 512 * n : 512 * (n + 1)],
                    centT_sb[:, 0, :],
                    featsT_sb[:, 0, 512 * n : 512 * (n + 1)],
                    start=True,
                    stop=False,
                )
            eng.wait_ge(fsems[0], 16)
            for n in (2, 3):
                eng.matmul(
                    ps_aff[:, 512 * n : 512 * (n + 1)],
                    centT_sb[:, 0, :],
                    featsT_sb[:, 0, 512 * n : 512 * (n + 1)],
                    start=True,
                    stop=False,
                )
            for k in range(1, 7):
                eng.wait_ge(fsems[k], 16)
                for n in range(4):
                    eng.matmul(
                        ps_aff[:, 512 * n : 512 * (n + 1)],
                        centT_sb[:, k, :],
                        featsT_sb[:, k, 512 * n : 512 * (n + 1)],
                        start=False,
                        stop=False,
                    )
            eng.wait_ge(fsems[7], 16)
            for n in range(4):
                eng.matmul(
                    ps_aff[:, 512 * n : 512 * (n + 1)],
                    centT_sb[:, 7, :],
                    featsT_sb[:, 7, 512 * n : 512 * (n + 1)],
                    start=False,
                    stop=True,
                ).then_inc(pe_sem, 1)

            # transposes of e0t into token-major (4 chunks of 128 per block);
            # block 3 goes to ps_c (free until C(0)) so it needn't wait for
            # any copy; block 2 reuses ps_tp after ACT's c0 copy
            def tp_block(j):
                buf = (ps_tp, ps_tp2, ps_tp, ps_c)[j]
                eng.wait_ge(act_sem, j + 1)       # exp block j done
                if j == 2:
                    eng.wait_ge(act_sem, 5)       # c0 freed ps_tp
                for t in range(4 * j, 4 * j + 4):
                    mm = eng.transpose(
                        buf[:, 128 * (t % 4) : 128 * (t % 4 + 1)],
                        e0t_sb[:, 128 * t : 128 * (t + 1)],
                        ident_sb[:],
                    )
                mm.then_inc(pe_sem, 1)

            tp_block(0)
            tp_block(1)

            # iteration-0 R-step (needs full exp + v0)
            eng.wait_ge(act_sem, A_EXP)
            eng.wait_ge(vsem, 16)
            for t in range(16):
                mm = eng.matmul(
                    ps_r[:, 2 * t : 2 * (t + 1)],
                    e0t_sb[:, 128 * t : 128 * (t + 1)],
                    v_sb[:],
                    start=True,
                    stop=True,
                )
            mm.then_inc(pe_sem, 1)

            tp_block(3)
            tp_block(2)

            for k in range(ITERS):
                if k > 0:
                    eng.wait_ge(dve_sem, V_V(k - 1))
                    for t in range(16):
                        mm = eng.matmul(
                            ps_r[:, 2 * t : 2 * (t + 1)],
                            e0t_sb[:, 128 * t : 128 * (t + 1)],
                            v_sb[:],
                            start=True,
                            stop=True,
                        )
                    mm.then_inc(pe_sem, 1)

                if k < ITERS - 1:
                    if k == 0:
                        eng.wait_ge(act_sem, 6)   # c0 + c2 landed
                        eng.wait_ge(cpsem, 2)     # c1 + c3 landed
                    eng.wait_ge(dve_sem, V_W(k))
                    for t in range(16):
                        mm = eng.matmul(
                            ps_c[:, 0:2],
                            e0tm_sb[:, t, :],
                            w_sb[:, t, :],
                            start=(t == 0),
                            stop=(t == 15),
                        )
                    mm.then_inc(pe_sem, 1)

        # ---------------- gpsimd (Pool): DMA queue 3 + RDMA butterfly ----
        @block.gpsimd
        def _(eng):
            eng.load_library(library_config.remote_dma)
            src_ap = centT_in.ap().rearrange("(k p) e -> p k e", p=128)
            with nc.allow_non_contiguous_dma(reason="8 strided 512B rows per partition"):
                eng.dma_start(out=centT_sb[:], in_=src_ap).then_inc(csem, 16)
            for k in (2, 5):
                eng.dma_start(
                    out=featsT_sb[:, k, :], in_=featsT_in[128 * k : 128 * (k + 1), :]
                ).then_inc(fsems[k], 16)

            def gen_preps(k):
                for r, (src, dst, delta) in enumerate(ROUNDS):
                    rdests = [None] * 8
                    rdests[delta] = (0, delta)
                    eng.remote_dma_broadcast(
                        out_ap=ex_sb[:, k, dst : dst + 1],
                        in_ap=ex_sb[:, k, src : src + 1],
                        remote_sem=rsems[r][k],
                        local_sem=lsem,
                        rdests=rdests,
                    )

            for k in range(min(PRELOAD, ITERS - 1)):
                gen_preps(k)
            for k in range(ITERS - 1):
                eng.wait_ge(dve_sem, V_CP(k))
                eng.trigger_dma(1).then_inc(tsem, 1)
                eng.wait_ge(dve_sem, V_S1(k))
                eng.trigger_dma(1).then_inc(tsem, 1)
                eng.wait_ge(dve_sem, V_S2(k))
                eng.trigger_dma(1).then_inc(tsem, 1)
                if k + PRELOAD <= ITERS - 2:
                    gen_preps(k + PRELOAD)
            # all sends complete before block drain
            eng.wait_ge(tsem, 3 * (ITERS - 1))

    # raw Bass doesn't run the extended-inst ISA codegen pass; without it the
    # NEFF compiler sees empty .instr ("ISA wrong length")
    from concourse.library_overlay import lower_extended_insts

    lower_extended_insts(nc)
    return nc


_CACHE = {}


def _get_nc():
    if "nc" not in _CACHE:
        _CACHE["nc"] = _build_nc()
    return _CACHE["nc"]


def make_in_maps(input_features, expert_centroids):
    feats = np.ascontiguousarray(
        np.asarray(input_features, dtype=np.float32).reshape(-1, D)
    )
    cent = np.asarray(expert_centroids, dtype=np.float32).reshape(SE, D)

    featsT = np.ascontiguousarray(feats.T)
    centT = np.ascontiguousarray(cent.T)
    ident = np.eye(128, dtype=np.float32)
    v0 = np.zeros((SE, 2), np.float32)
    v0[0:64, 0] = 1.0
    v0[64:128, 1] = 1.0

    in_maps = []
    for c in range(N_CORES):
        in_maps.append(
            {
                "featsT": np.ascontiguousarray(featsT[:, TOK * c : TOK * (c + 1)]),
                "centT": centT,
                "ident": ident,
                "v0": v0,
            }
        )
    return in_maps


def kernel(input_features: np.ndarray, expert_centroids: np.ndarray):
    in_maps = make_in_maps(input_features, expert_centroids)
    nc = _get_nc()
    res = run_bass_kernel_spmd(nc, in_maps, list(range(N_CORES)))

    afft = np.concatenate([res.results[c]["afft"] for c in range(N_CORES)], axis=1)
    # rfin[c][p, 2t+s] = R(token c*2048 + t*128 + p, slot s)
    rfin = np.stack([res.results[c]["rfin"] for c in range(N_CORES)])  # [8,128,32]
    r = rfin.reshape(N_CORES, 128, 16, 2).transpose(0, 2, 1, 3).reshape(N, 2)

    A = afft.reshape(KSLOT, E, N)
    lnr = np.log(r.astype(np.float64)).astype(np.float32)  # [N, 2]
    idx = np.empty((KSLOT, E, CAP), np.int32)
    vals = np.empty((KSLOT, E, CAP), np.float32)
    for k in range(KSLOT):
        Z = A[k] - lnr[:, k][None, :]
        for e in range(E):
            col = Z[e]
            part = np.sort(np.argpartition(-col, CAP - 1)[:CAP])
            order = part[np.argsort(-col[part], kind="stable")]
            idx[k, e] = order.astype(np.int32)
            vals[k, e] = A[k, e, order]
    return idx, vals


# revision 79
# speedup vs baseline: 7.2254x; 1.1844x over previous
"""Trainium2 Bass kernel for nn_BaseLayerGate (MoE balanced routing).

8 NeuronCores, data-parallel over tokens:
  - Each core owns a 2048-token shard. Affinity matmul aff^T = centT.T @ featsT
    on the tensor engine (fp32r: full fp32 precision at 1 cycle/row).
  - Sinkhorn (10 iters) in reciprocal-potential form:
      R_sum[n]  = sum_se E0[n,se] * V[se]      (PE matvec, V = slot-masked 1/C_sum)
      C_sum[se] = sum_n  E0[n,se] * W[n]       (PE matvec accum, W = 1/R_sum)
    The token-direction sum is global: per-expert partials are allreduced
    across the 8 cores with a 3-round XOR butterfly over remote DMA
    (remote_dma_broadcast with relative dests; descs pre-generated on the
    gpsimd SWDGE ring, fired by trigger_dma). 10 R-steps, 9 C-steps (the
    10th C-step is a uniform per-column shift and cannot change top-k
    ordering).
  - Outputs: aff^T (fp32) and the final R_sum; the host computes
    Z = aff - ln(R) (fp64 ln), does the per-expert top-k and gathers vals.
"""

import numpy as np

import concourse.bass as bass
from concourse import library_config, mybir
from concourse.bass_utils import run_bass_kernel_spmd


def _install_sim_topology_fallback():
    """The instruction simulator resolves remote-DMA routing through libndbg
    driver ioctls; in driverless containers those fail. Install a
    deterministic single-node topology so the simulator can model the
    XOR-relative remote DMAs. Any bijective nc mapping is equivalent for the
    XOR butterfly (on hardware the Q7 resolves relative dests itself, so this
    fallback never affects device execution)."""
    import concourse.libnrt as libnrt

    try:
        libnrt.get_trn2_nc_mapping()
        return
    except Exception:
        pass
    ncmap = {(d, i): i for d in range(16) for i in range(8)}
    ridmap = {d: d for d in range(16)}
    libnrt.get_trn2_nc_mapping = lambda: ncmap
    libnrt.get_device_id_to_routing_id_mapping = lambda: ridmap
    for f in (
        libnrt.nc_to_real_nc,
        libnrt.CoreAddress.from_pnc,
        libnrt._real_nc_to_pnc_mapping,
        libnrt.get_routing_id_to_device_id_mapping,
    ):
        try:
            f.cache_clear()
        except AttributeError:
            pass
    import concourse.bass_interp as bass_interp

    bass_interp.get_device_id_to_routing_id_mapping = (
        libnrt.get_device_id_to_routing_id_mapping
    )


_install_sim_topology_fallback()

N_CORES = 8
N = 16384
D = 1024
KSLOT = 2
E = 64
SE = KSLOT * E
CAP = N // E
TOK = N // N_CORES
ITERS = 10

F32 = mybir.dt.float32
F32R = mybir.dt.float32r

# ---- semaphore schedule ----------------------------------------------------
# pe_sem: aff col-block n done = 1+n (n=0..3) | tp0=5 | tp1=6 | R(0)=7 |
# tp3=8 | tp2=9 | C(k)=10+2k | R(k>=1)=9+2k
def P_B(n):
    return 1 + n
def P_R(k):
    return 7 if k == 0 else 9 + 2 * k
def P_C(k):
    return 10 + 2 * k
P_LAST_R = P_R(ITERS - 1)                      # 27
P_TP = [5, 6, 9, 8]  # indexed by block: tp2 runs after tp3 on PE

# act_sem: exp blocks 1..4 | c0=5 | c2=6 | afft copy 7 | rfin copy 8
A_EXP = 4
A_AFFT = 7
A_RFIN = 8
# cpsem (DVE copies): c1=1 | c3=2

# dve_sem per iter k (0..8):
# recipW=6k+1, cpart=6k+2, s1=6k+3, s2=6k+4, csum=6k+5, v=6k+6
def V_W(k):
    return 6 * k + 1
def V_CP(k):
    return 6 * k + 2
def V_S1(k):
    return 6 * k + 3
def V_S2(k):
    return 6 * k + 4
def V_CS(k):
    return 6 * k + 5
def V_V(k):
    return 6 * k + 6

# butterfly rounds: (src slot, dst slot, xor distance, rdest slot index)
ROUNDS = [(0, 1, 1), (2, 3, 2), (4, 5, 4)]
PRELOAD = 4   # iterations of preps generated before the trigger loop


def _build_nc():
    nc = bass.Bass()

    # hi/lo bf16 split of featsT/centT: aff = hi*hi + hi*lo + lo*hi (the
    # dropped lo*lo term is ~2^-18 relative) at 3 bf16 passes vs fp32's 4
    BF16 = mybir.dt.bfloat16
    featsT_in = nc.declare_dram_parameter("featsT", [D, 2, TOK], BF16, isOutput=False)
    centT_in = nc.declare_dram_parameter("centT", [D, 2 * SE], BF16, isOutput=False)
    v0_in = nc.declare_dram_parameter("v0", [SE, 2], F32, isOutput=False)
    ident_in = nc.declare_dram_parameter("ident", [128, 128], F32, isOutput=False)

    aff_out = nc.declare_dram_parameter("afft", [SE, TOK], F32, isOutput=True)
    rfin_out = nc.declare_dram_parameter("rfin", [128, 32], F32, isOutput=True)

    from contextlib import ExitStack
    es = ExitStack()
    featsT_sb = es.enter_context(nc.sbuf_tensor("featsT_sb", [128, 8, 2, TOK], BF16))
    centT_sb = es.enter_context(nc.sbuf_tensor("centT_sb", [128, 8, 2, SE], BF16))
    e0t_sb = es.enter_context(nc.sbuf_tensor("e0t_sb", [128, TOK], F32))
    afft_sb = es.enter_context(nc.sbuf_tensor("afft_sb", [128, TOK], F32))
    e0tm_sb = es.enter_context(nc.sbuf_tensor("e0tm_sb", [128, 16, 128], F32))
    ident_sb = es.enter_context(nc.sbuf_tensor("ident_sb", [128, 128], F32))
    escr_sb = es.enter_context(nc.sbuf_tensor("escr_sb", [128, 1], F32))
    v_sb = es.enter_context(nc.sbuf_tensor("v_sb", [128, 2], F32))
    w_sb = es.enter_context(nc.sbuf_tensor("w_sb", [128, 16, 2], F32))
    # exchange slots, one bank per iteration (no reuse -> no WAR hazards):
    # slots: 0=own cpart, 1=recv d1, 2=s1, 3=recv d2, 4=s2, 5=recv d4
    ex_sb = es.enter_context(nc.sbuf_tensor("ex_sb", [128, ITERS - 1, 6], F32))
    csum_sb = es.enter_context(nc.sbuf_tensor("csum_sb", [128, 1], F32))
    rfin_sb = es.enter_context(nc.sbuf_tensor("rfin_sb", [128, 32], F32))
    ps_aff = es.enter_context(nc.psum_tensor("ps_aff", [128, TOK], F32))
    ps_tp = es.enter_context(nc.psum_tensor("ps_tp", [128, 512], F32))
    ps_tp2 = es.enter_context(nc.psum_tensor("ps_tp2", [128, 512], F32))
    ps_r = es.enter_context(nc.psum_tensor("ps_r", [128, 512], F32))
    ps_c = es.enter_context(nc.psum_tensor("ps_c", [128, 512], F32))
    block = es.enter_context(nc.Block())
    csem = es.enter_context(nc.semaphore("csem"))      # centT
    vsem = es.enter_context(nc.semaphore("vsem"))      # v0
    isem = es.enter_context(nc.semaphore("isem"))      # ident
    out_sem = es.enter_context(nc.semaphore("out_sem"))
    fsems = [es.enter_context(nc.semaphore(f"fsem{k}")) for k in range(8)]
    f0a_sem = es.enter_context(nc.semaphore("f0a_sem"))
    pe_sem = es.enter_context(nc.semaphore("pe_sem"))
    act_sem = es.enter_context(nc.semaphore("act_sem"))
    dve_sem = es.enter_context(nc.semaphore("dve_sem"))
    lsem = es.enter_context(nc.semaphore("lsem"))
    tsem = es.enter_context(nc.semaphore("tsem"))  # trigger completion (normal sem)
    # one remote sem per (round, iteration): single update, single wait
    rsems = [
        [es.enter_context(nc.semaphore(f"rsem{r}_{k}")) for k in range(ITERS - 1)]
        for r in range(3)
    ]
    cpsem = es.enter_context(nc.semaphore("cpsem"))

    with es:
        # ---------------- sync engine (SP): DMA queue 1 ----------------
        @block.sync
        def _(eng):
            # chunk 0 split into token halves so the matmul starts ~1.6us
            # earlier (column block n covers tokens 512n..512n+511)
            with nc.allow_non_contiguous_dma(reason="hi+lo token-half slices"):
                eng.dma_start(
                    out=featsT_sb[:, 0, :, 0:1024], in_=featsT_in[0:128, :, 0:1024]
                ).then_inc(f0a_sem, 16)
                eng.dma_start(
                    out=featsT_sb[:, 0, :, 1024:2048],
                    in_=featsT_in[0:128, :, 1024:2048],
                ).then_inc(fsems[0], 16)
            eng.dma_start(
                out=featsT_sb[:, 3, :, :], in_=featsT_in[128 * 3 : 128 * 4, :, :]
            ).then_inc(fsems[3], 16)
            eng.dma_start(out=v_sb[:], in_=v0_in[:]).then_inc(vsem, 16)
            eng.dma_start(
                out=featsT_sb[:, 6, :, :], in_=featsT_in[128 * 6 : 128 * 7, :, :]
            ).then_inc(fsems[6], 16)

            eng.wait_ge(act_sem, A_AFFT)
            eng.dma_start(out=aff_out[:], in_=afft_sb[:]).then_inc(out_sem, 16)
            eng.wait_ge(out_sem, 32)

        # ---------------- vector engine (DVE): e0tm copies + sinkhorn ----
        @block.vector
        def _(eng):
            eng.wait_ge(pe_sem, P_TP[1])
            eng.tensor_copy(
                e0tm_sb.ap().rearrange("p t c -> p (t c)")[:, 512:1024],
                ps_tp2[:, 0:512],
            ).then_inc(cpsem, 1)
            eng.wait_ge(pe_sem, P_TP[3])
            eng.tensor_copy(
                e0tm_sb.ap().rearrange("p t c -> p (t c)")[:, 1536:2048],
                ps_c[:, 0:512],
            ).then_inc(cpsem, 1)
            for k in range(ITERS - 1):
                eng.wait_ge(pe_sem, P_R(k))
                eng.reciprocal(
                    w_sb.ap().rearrange("p t s -> p (t s)"), ps_r[:, 0:32]
                ).then_inc(dve_sem, 1)
                eng.wait_ge(pe_sem, P_C(k))
                eng.tensor_copy(ex_sb[0:64, k, 0:1], ps_c[0:64, 0:1])
                eng.tensor_copy(ex_sb[64:128, k, 0:1], ps_c[64:128, 1:2]).then_inc(
                    dve_sem, 1
                )
                eng.wait_ge(rsems[0][k], 2)
                eng.wait_ge(dve_sem, V_CP(k))
                eng.tensor_add(
                    ex_sb[:, k, 2:3], ex_sb[:, k, 0:1], ex_sb[:, k, 1:2]
                ).then_inc(dve_sem, 1)
                eng.wait_ge(rsems[1][k], 2)
                eng.wait_ge(dve_sem, V_S1(k))
                eng.tensor_add(
                    ex_sb[:, k, 4:5], ex_sb[:, k, 2:3], ex_sb[:, k, 3:4]
                ).then_inc(dve_sem, 1)
                eng.wait_ge(rsems[2][k], 2)
                eng.wait_ge(dve_sem, V_S2(k))
                eng.tensor_add(
                    csum_sb[:], ex_sb[:, k, 4:5], ex_sb[:, k, 5:6]
                ).then_inc(dve_sem, 1)
                eng.wait_ge(dve_sem, V_CS(k))
                eng.reciprocal(v_sb[0:64, 0:1], csum_sb[0:64, :])
                eng.reciprocal(v_sb[64:128, 1:2], csum_sb[64:128, :]).then_inc(
                    dve_sem, 1
                )

        # ---------------- scalar engine (ACT): DMA queue 2 + exp/copies --
        @block.scalar
        def _(eng):
            eng.dma_start(out=ident_sb[:], in_=ident_in[:]).then_inc(isem, 16)
            eng.dma_start(
                out=featsT_sb[:, 1, :, :], in_=featsT_in[128 * 1 : 128 * 2, :, :]
            ).then_inc(fsems[1], 16)
            # preload the Exp activation table while the matmul runs
            eng.activation(
                escr_sb[:], ident_sb[:, 0:1], mybir.ActivationFunctionType.Exp
            )
            for k in (4, 7):
                eng.dma_start(
                    out=featsT_sb[:, k, :, :],
                    in_=featsT_in[128 * k : 128 * (k + 1), :, :],
                ).then_inc(fsems[k], 16)

            for bkl in range(4):
                eng.wait_ge(pe_sem, P_B(bkl))
                eng.activation(
                    e0t_sb[:, 512 * bkl : 512 * (bkl + 1)],
                    ps_aff[:, 512 * bkl : 512 * (bkl + 1)],
                    mybir.ActivationFunctionType.Exp,
                ).then_inc(act_sem, 1)
            for j in (0, 2):
                eng.wait_ge(pe_sem, P_TP[j])
                eng.activation(
                    e0tm_sb.ap().rearrange("p t c -> p (t c)")[:, 512 * j : 512 * (j + 1)],
                    ps_tp[:, 0:512],
                    mybir.ActivationFunctionType.Copy,
                ).then_inc(act_sem, 1)
            eng.activation(
                afft_sb[:], ps_aff[:, 0:TOK], mybir.ActivationFunctionType.Copy
            ).then_inc(act_sem, 1)
            eng.wait_ge(pe_sem, P_LAST_R)
            eng.activation(
                rfin_sb[:], ps_r[:, 0:32], mybir.ActivationFunctionType.Copy
            ).then_inc(act_sem, 1)
            eng.wait_ge(act_sem, A_RFIN)
            eng.dma_start(out=rfin_out[:], in_=rfin_sb[:]).then_inc(out_sem, 16)

        # ---------------- tensor engine (PE) ----------------
        @block.tensor
        def _(eng):
            # warm the PE p-state ramp on ident transposes while chunk 0 loads
            # (targets: unused psum regions, one write each)
            eng.wait_ge(isem, 16)
            for wbuf, lo in ((ps_r, 128), (ps_r, 256), (ps_r, 384)):
                eng.transpose(wbuf[:, lo : lo + 128], ident_sb[:], ident_sb[:])

            # affinity matmul fp32 (exact): chunk 0 in token halves (starts as
            # soon as the first half lands), chunks 1-6 k-outer (pipelines
            # with the feats DMAs), then chunk 7 per column block so the
            # exp / transpose chain can chase the matmul tail block by block
            # products per (chunk, block): hi*hi, hi*lo, lo*hi
            def mm3(k, n, start, stop):
                for i, (hc, hf) in enumerate(((0, 0), (0, 1), (1, 0))):
                    mm = eng.matmul(
                        ps_aff[:, 512 * n : 512 * (n + 1)],
                        centT_sb[:, k, hc, :],
                        featsT_sb[:, k, hf, 512 * n : 512 * (n + 1)],
                        start=start and i == 0,
                        stop=stop and i == 2,
                    )
                return mm

            eng.wait_ge(csem, 16)
            eng.wait_ge(f0a_sem, 16)
            for n in (0, 1):
                mm3(0, n, True, False)
            eng.wait_ge(fsems[0], 16)
            for n in (2, 3):
                mm3(0, n, True, False)
            for k in range(1, 7):
                eng.wait_ge(fsems[k], 16)
                for n in range(4):
                    mm3(k, n, False, False)
            eng.wait_ge(fsems[7], 16)
            for n in range(4):
                mm3(7, n, False, True).then_inc(pe_sem, 1)

            # transposes of e0t into token-major (4 chunks of 128 per block);
            # block 3 goes to ps_c (free until C(0)) so it needn't wait for
            # any copy; block 2 reuses ps_tp after ACT's c0 copy
            def tp_block(j):
                buf = (ps_tp, ps_tp2, ps_tp, ps_c)[j]
                eng.wait_ge(act_sem, j + 1)       # exp block j done
                if j == 2:
                    eng.wait_ge(act_sem, 5)       # c0 freed ps_tp
                for t in range(4 * j, 4 * j + 4):
                    mm = eng.transpose(
                        buf[:, 128 * (t % 4) : 128 * (t % 4 + 1)],
                        e0t_sb[:, 128 * t : 128 * (t + 1)],
                        ident_sb[:],
                    )
                mm.then_inc(pe_sem, 1)

            tp_block(0)
            tp_block(1)

            # iteration-0 R-step (needs full exp + v0)
            eng.wait_ge(act_sem, A_EXP)
            eng.wait_ge(vsem, 16)
            for t in range(16):
                mm = eng.matmul(
                    ps_r[:, 2 * t : 2 * (t + 1)],
                    e0t_sb[:, 128 * t : 128 * (t + 1)],
                    v_sb[:],
                    start=True,
                    stop=True,
                )
            mm.then_inc(pe_sem, 1)

            tp_block(3)
            tp_block(2)

            for k in range(ITERS):
                if k > 0:
                    eng.wait_ge(dve_sem, V_V(k - 1))
                    for t in range(16):
                        mm = eng.matmul(
                            ps_r[:, 2 * t : 2 * (t + 1)],
                            e0t_sb[:, 128 * t : 128 * (t + 1)],
                            v_sb[:],
                            start=True,
                            stop=True,
                        )
                    mm.then_inc(pe_sem, 1)

                if k < ITERS - 1:
                    if k == 0:
                        eng.wait_ge(act_sem, 6)   # c0 + c2 landed
                        eng.wait_ge(cpsem, 2)     # c1 + c3 landed
                    eng.wait_ge(dve_sem, V_W(k))
                    for t in range(16):
                        mm = eng.matmul(
                            ps_c[:, 0:2],
                            e0tm_sb[:, t, :],
                            w_sb[:, t, :],
                            start=(t == 0),
                            stop=(t == 15),
                        )
                    mm.then_inc(pe_sem, 1)

        # ---------------- gpsimd (Pool): DMA queue 3 + RDMA butterfly ----
        @block.gpsimd
        def _(eng):
            eng.load_library(library_config.remote_dma)
            src_ap = centT_in.ap().rearrange("(k p) e -> p k e", p=128)
            with nc.allow_non_contiguous_dma(reason="8 strided 512B rows per partition"):
                eng.dma_start(
                    out=centT_sb.ap().rearrange("p k h e -> p k (h e)"), in_=src_ap
                ).then_inc(csem, 16)
            for k in (2, 5):
                eng.dma_start(
                    out=featsT_sb[:, k, :, :],
                    in_=featsT_in[128 * k : 128 * (k + 1), :, :],
                ).then_inc(fsems[k], 16)

            def gen_preps(k):
                for r, (src, dst, delta) in enumerate(ROUNDS):
                    rdests = [None] * 8
                    rdests[delta] = (0, delta)
                    eng.remote_dma_broadcast(
                        out_ap=ex_sb[:, k, dst : dst + 1],
                        in_ap=ex_sb[:, k, src : src + 1],
                        remote_sem=rsems[r][k],
                        local_sem=lsem,
                        rdests=rdests,
                    )

            for k in range(min(PRELOAD, ITERS - 1)):
                gen_preps(k)
            for k in range(ITERS - 1):
                eng.wait_ge(dve_sem, V_CP(k))
                eng.trigger_dma(1).then_inc(tsem, 1)
                eng.wait_ge(dve_sem, V_S1(k))
                eng.trigger_dma(1).then_inc(tsem, 1)
                eng.wait_ge(dve_sem, V_S2(k))
                eng.trigger_dma(1).then_inc(tsem, 1)
                if k + PRELOAD <= ITERS - 2:
                    gen_preps(k + PRELOAD)
            # all sends complete before block drain
            eng.wait_ge(tsem, 3 * (ITERS - 1))

    # raw Bass doesn't run the extended-inst ISA codegen pass; without it the
    # NEFF compiler sees empty .instr ("ISA wrong length")
    from concourse.library_overlay import lower_extended_insts

    lower_extended_insts(nc)
    return nc


_CACHE = {}


def _get_nc():
    if "nc" not in _CACHE:
        _CACHE["nc"] = _build_nc()
    return _CACHE["nc"]


def _hilo(x):
    """bf16 hi/lo split along a new axis 1: x ~= hi + lo."""
    import ml_dtypes

    hi = x.astype(ml_dtypes.bfloat16)
    lo = (x - hi.astype(np.float32)).astype(ml_dtypes.bfloat16)
    return np.ascontiguousarray(np.stack([hi, lo], axis=1))


def make_in_maps(input_features, expert_centroids):
    feats = np.ascontiguousarray(
        np.asarray(input_features, dtype=np.float32).reshape(-1, D)
    )
    cent = np.asarray(expert_centroids, dtype=np.float32).reshape(SE, D)

    featsT2 = _hilo(np.ascontiguousarray(feats.T))        # [D, 2, N]
    centT2 = _hilo(np.ascontiguousarray(cent.T))          # [D, 2, SE]
    centT2 = centT2.reshape(D, 2 * SE)
    ident = np.eye(128, dtype=np.float32)
    v0 = np.zeros((SE, 2), np.float32)
    v0[0:64, 0] = 1.0
    v0[64:128, 1] = 1.0

    in_maps = []
    for c in range(N_CORES):
        in_maps.append(
            {
                "featsT": np.ascontiguousarray(
                    featsT2[:, :, TOK * c : TOK * (c + 1)]
                ),
                "centT": centT2,
                "ident": ident,
                "v0": v0,
            }
        )
    return in_maps


def kernel(input_features: np.ndarray, expert_centroids: np.ndarray):
    in_maps = make_in_maps(input_features, expert_centroids)
    nc = _get_nc()
    res = run_bass_kernel_spmd(nc, in_maps, list(range(N_CORES)))

    afft = np.concatenate([res.results[c]["afft"] for c in range(N_CORES)], axis=1)
    # rfin[c][p, 2t+s] = R(token c*2048 + t*128 + p, slot s)
    rfin = np.stack([res.results[c]["rfin"] for c in range(N_CORES)])  # [8,128,32]
    r = rfin.reshape(N_CORES, 128, 16, 2).transpose(0, 2, 1, 3).reshape(N, 2)

    A = afft.reshape(KSLOT, E, N)
    lnr = np.log(r.astype(np.float64)).astype(np.float32)  # [N, 2]
    idx = np.empty((KSLOT, E, CAP), np.int32)
    vals = np.empty((KSLOT, E, CAP), np.float32)
    for k in range(KSLOT):
        Z = A[k] - lnr[:, k][None, :]
        for e in range(E):
            col = Z[e]
            part = np.sort(np.argpartition(-col, CAP - 1)[:CAP])
            order = part[np.argsort(-col[part], kind="stable")]
            idx[k, e] = order.astype(np.int32)
            vals[k, e] = A[k, e, order]
    return idx, vals
